# revision 1
# baseline (speedup 1.0000x reference)
"""Trainium2 Bass kernel for nn_Attention (B=1, C=64, 12x12x12 spatial, 32 heads, head_dim=2).

Sharding: 32 heads split across 8 cores (4 heads/core). Each core computes
qkv projection for its heads, head-local attention (flash-style: S^T chunks
-> exp on ScalarE -> U/Z accumulation via matmul with V'=[V,1]), divides,
then applies its slice of w_proj rows to produce a partial output summed on
the host (tensor-parallel unshard) with bias/8 folded per core.

Uses bacc.Bacc (not plain Bass): its compile() runs
move_matmul_waits_to_ldweights + generate_event_semaphores, which the
TRN2 one-wait-per-instruction ISA constraint requires for Tile kernels.

Scheduling notes: Tile's static scheduler keeps per-engine creation order,
so overlap is structured by emission order — qkv for the second query tile
is emitted inside the first tile's key loop (PE is idle there; ScalarE exp
is the bottleneck), and the first tile's divide/proj are emitted before the
second tile's loop so they run under it.

Self-contained: hardcodes all shapes.
"""

import numpy as np
import ml_dtypes

import concourse.bass as bass
import concourse.bacc as bacc
import concourse.mybir as mybir
from concourse import tile
from concourse.bass_utils import run_bass_kernel_spmd

C = 64
N = 1728  # 12*12*12
NCORES = 8
HLOC = 4          # heads per core
SCALE = float(2.0 ** -0.5)

# key chunks: 13x128 + 64
KCS = [(i * 128, 128) for i in range(13)] + [(1664, 64)]
NKC = len(KCS)
# query tiles: big first tile, small second so the un-overlappable tail
# (reciprocal is FD-bound at 8 cyc/elem) is short
QTS = [(0, 1024), (1024, 704)]
# token chunks for proj; chunks 0..8 lie fully inside query tile 0
TCS = [(i * 108, 108) for i in range(16)]
TC_SPLIT = 9

F32 = mybir.dt.float32
BF16 = mybir.dt.bfloat16


def _sub_mms(qn):
    out = []
    o = 0
    while o < qn:
        n = min(512, qn - o)
        out.append((o, n))
        o += n
    return out


def build_nc():
    nc = bacc.Bacc(None)

    x2 = nc.declare_dram_parameter("x2", [C, N], BF16, isOutput=False)
    wq = nc.declare_dram_parameter("wq", [C, 2 * HLOC], BF16, isOutput=False)
    wk = nc.declare_dram_parameter("wk", [C, 2 * HLOC], BF16, isOutput=False)
    wv = nc.declare_dram_parameter("wv", [C, 2 * HLOC], BF16, isOutput=False)
    wp = nc.declare_dram_parameter("wp", [2 * HLOC + 1, C], F32, isOutput=False)
    y = nc.declare_dram_parameter("y", [N, C], F32, isOutput=True)

    with tile.TileContext(nc) as tc:
        with (
            tc.tile_pool(name="const", bufs=1) as cpool,
            tc.tile_pool(name="epool", bufs=5) as epool,
            tc.tile_pool(name="upool", bufs=2) as upool,
            tc.tile_pool(name="ps_s", bufs=3, space=bass.MemorySpace.PSUM) as ps_s,
            tc.tile_pool(name="ps_u", bufs=1, space=bass.MemorySpace.PSUM) as ps_u,
        ):
            x_st = cpool.tile([C, N], BF16, name="x_st")
            x_sb = cpool.tile([C, N], BF16, name="x_sb")
            wq_st = cpool.tile([C, 2 * HLOC], BF16, name="wq_st")
            wq_sb = cpool.tile([C, 2 * HLOC], BF16, name="wq_sb")
            wk_st = cpool.tile([C, 2 * HLOC], BF16, name="wk_st")
            wk_sb = cpool.tile([C, 2 * HLOC], BF16, name="wk_sb")
            wv_st = cpool.tile([C, 2 * HLOC], BF16, name="wv_st")
            wv_sb = cpool.tile([C, 2 * HLOC], BF16, name="wv_sb")
            wp_st = cpool.tile([2 * HLOC + 1, C], F32, name="wp_st")
            wp_sb = cpool.tile([2 * HLOC + 1, C], F32, name="wp_sb")
            qT = cpool.tile([128, N], BF16, name="qT")
            kT = cpool.tile([128, N], BF16, name="kT")
            vp = cpool.tile([128, NKC * 3 * HLOC], BF16, name="vp")
            ot = cpool.tile([2 * HLOC + 1, N], F32, name="ot")
            ybig = cpool.tile([128, len(TCS) * C], F32, name="ybig")
            ybv = ybig[:].rearrange("p (t c) -> p t c", c=C)

            # x DMA first (it gates everything); stage through one DVE copy
            # each so consumers wait on a single engine semaphore instead of
            # one per DMA queue.
            nc.sync.dma_start(out=x_st[:], in_=x2[:])
            nc.sync.dma_start(out=wv_st[:], in_=wv[:])
            nc.sync.dma_start(out=wq_st[:], in_=wq[:])
            nc.sync.dma_start(out=wk_st[:], in_=wk[:])
            nc.sync.dma_start(out=wp_st[:], in_=wp[:])
            nc.vector.tensor_copy(x_sb[:, 0:1024], x_st[:, 0:1024])
            nc.vector.tensor_copy(x_sb[:, 1024:N], x_st[:, 1024:N])
            nc.vector.tensor_copy(wv_sb[:], wv_st[:])
            nc.vector.tensor_copy(wq_sb[:], wq_st[:])
            nc.vector.tensor_copy(wk_sb[:], wk_st[:])
            nc.vector.tensor_copy(wp_sb[:], wp_st[:])

            # ones row for proj bias (rows 0..7 overwritten by attention out)
            nc.gpsimd.memset(ot[:, :], 1.0)
            # ones column per head in V' ([128, kc, h, 3], col 2 = 1.0)
            vp_v = vp[:].rearrange("p (a b c) -> p a b c", b=HLOC, c=3)
            nc.gpsimd.memset(vp_v[:, :, :, 2:3], 1.0)

            # ---- V': all 14 key chunks' V rows packed into ONE psum tile
            # (emitted as pre_u of kc0 — only U matmuls need it) ----
            def emit_vprime():
                psv = ps_s.tile([128, 1024], F32, tag="s", name="ps_v")
                for kc, (ko, kn) in enumerate(KCS):
                    nc.tensor.matmul(
                        psv[:kn, 8 * kc : 8 * kc + 2 * HLOC],
                        x_sb[:, ko : ko + kn],
                        wv_sb[:, 0 : 2 * HLOC],
                        start=True, stop=True,
                    )
                vsrc = psv[:, 0 : 8 * NKC].rearrange(
                    "p (kc h d) -> p kc h d", h=HLOC, d=2
                )
                nc.vector.tensor_copy(vp_v[:, :, :, 0:2], vsrc)

            def qkv_tile(w_sb, dst, off, qn, heads=range(HLOC)):
                """Per-head matmuls (rows at partitions 32h) + per-head copy."""
                ps = ps_s.tile([128, 1024], F32, tag="s", name="ps_qkv")
                for h in heads:
                    for (o, n_) in _sub_mms(qn):
                        nc.tensor.matmul(
                            ps[32 * h : 32 * h + 2, o : o + n_],
                            w_sb[:, 2 * h : 2 * h + 2],
                            x_sb[:, off + o : off + o + n_],
                            start=True, stop=True,
                            tile_position=(0, 32 * h),
                        )
                    nc.vector.tensor_copy(
                        dst[32 * h : 32 * h + 2, off : off + qn],
                        ps[32 * h : 32 * h + 2, :qn],
                    )

            # q half 0 / first k cols are emitted per-head just before each
            # head's first S matmul (pre_s of kc0) so exp h0 starts ASAP
            def pre_s0(kc, h):
                if kc == 0:
                    qkv_tile(wq_sb, qT, 0, 1024, heads=[h])
                    qkv_tile(wk_sb, kT, 0, 512, heads=[h])

            def pre_u0(kc):
                if kc == 0:
                    emit_vprime()

            def divide_and_store(pu, qo, qn, last=False):
                """O^T rows 2h+d of `ot` <- U rows / Z row (per head)."""
                if last:
                    # final tile: read PSUM directly, no next user of the slot
                    usrc = pu[:, :qn]
                else:
                    u_sb = upool.tile([128, 1024], F32, tag="u_sb", name="u_sb")
                    nc.vector.tensor_copy(u_sb[:, :qn], pu[:, :qn])
                    usrc = u_sb[:, :qn]
                zrec = upool.tile([128, 1024], F32, tag="zrec", name="zrec")
                nc.vector.reciprocal(zrec[:, :qn], usrc)
                zz = upool.tile([128, 1024], F32, tag="zz", name="zz")
                zzv_ = zz[:, :qn].rearrange("(h g) f -> h g f", g=32)
                zrv_ = zrec[:, :qn].rearrange("(h g) f -> h g f", g=32)
                nc.sync.dma_start(out=zzv_[:, 0, :], in_=zrv_[:, 2, :])
                nc.gpsimd.dma_start(out=zzv_[:, 1, :], in_=zrv_[:, 2, :])
                osp = upool.tile([128, 1024], F32, tag="osp", name="osp")
                nc.vector.tensor_mul(osp[:, :qn], usrc, zz[:, :qn])
                ospv = osp[:, :qn].rearrange("(h g) f -> h g f", g=32)
                otv = ot[0 : 2 * HLOC, qo : qo + qn].rearrange("(h g) f -> h g f", g=2)
                nc.sync.dma_start(out=otv[:, 0, :], in_=ospv[:, 0, :])
                nc.gpsimd.dma_start(out=otv[:, 1, :], in_=ospv[:, 1, :])

            def proj_chunks(ts_):
                for t in ts_:
                    to, tn = TCS[t]
                    py = ps_s.tile([128, 1024], F32, tag="s", name="py")
                    nc.tensor.matmul(
                        py[:tn, 0:C], ot[:, to : to + tn], wp_sb[:],
                        start=True, stop=True,
                    )
                    nc.vector.tensor_copy(ybv[:tn, t, :], py[:tn, 0:C])

            def main_loop(qo, qn, boundary_work, pre_s=None, pre_u=None):
                pu = ps_u.tile([128, 1024], F32, tag="pu", name="pu")
                for kc, (ko, kn) in enumerate(KCS):
                    es = []
                    for h in range(HLOC):
                        if pre_s is not None:
                            pre_s(kc, h)
                        ps = ps_s.tile([128, 1024], F32, tag="s", name="ps_att")
                        for (o, n_) in _sub_mms(qn):
                            nc.tensor.matmul(
                                ps[:kn, o : o + n_],
                                kT[32 * h : 32 * h + 2, ko : ko + kn],
                                qT[32 * h : 32 * h + 2, qo + o : qo + o + n_],
                                start=True, stop=True,
                                tile_position=(32 * h, 0),
                            )
                        e = epool.tile([128, 1024], BF16, tag="e", name="e")
                        nc.scalar.activation(
                            e[:kn, :qn], ps[:kn, :qn],
                            mybir.ActivationFunctionType.Exp, scale=SCALE,
                        )
                        es.append(e)
                    if pre_u is not None:
                        pre_u(kc)
                    for h in range(HLOC):
                        for (o, n_) in _sub_mms(qn):
                            nc.tensor.matmul(
                                pu[32 * h : 32 * h + 3, o : o + n_],
                                vp_v[:kn, kc, h, :],
                                es[h][:kn, o : o + n_],
                                start=(kc == 0), stop=(kc == NKC - 1),
                                tile_position=(0, 32 * h),
                            )
                    work = boundary_work.get(kc)
                    if work:
                        work()
                return pu

            # qt0 loop: remaining qkv emitted at key-loop boundaries, one
            # small piece per boundary (PE slack under the ACT-bound loop)
            bw0 = {
                0: lambda: qkv_tile(wk_sb, kT, 512, 512),
                1: lambda: qkv_tile(wq_sb, qT, 1024, 704, heads=[0, 1]),
                2: lambda: qkv_tile(wq_sb, qT, 1024, 704, heads=[2, 3]),
                3: lambda: qkv_tile(wk_sb, kT, 1024, 704, heads=[0, 1]),
                4: lambda: qkv_tile(wk_sb, kT, 1024, 704, heads=[2, 3]),
            }
            pu0 = main_loop(0, 1024, bw0, pre_s=pre_s0, pre_u=pre_u0)
            divide_and_store(pu0, 0, 1024)

            # qt1 loop: qt0's proj + first y DMA emitted at late boundaries
            # (after qt0's divide chain has drained on DVE/DMA)
            def y_dma0():
                yv0 = y[0 : TC_SPLIT * 108, :].rearrange("(t i) c -> i t c", i=108)
                nc.sync.dma_start(out=yv0, in_=ybv[:108, 0:TC_SPLIT, :])

            bw1 = {kc: (lambda t=kc - 3: proj_chunks([t])) for kc in range(3, 12)}
            bw1[12] = y_dma0
            pu1 = main_loop(1024, 704, bw1)
            divide_and_store(pu1, 1024, 704, last=True)
            proj_chunks(range(TC_SPLIT, len(TCS)))
            yv1 = y[TC_SPLIT * 108 :, :].rearrange("(t i) c -> i t c", i=108)
            nc.sync.dma_start(out=yv1, in_=ybv[:108, TC_SPLIT:, :])

    return nc


_NC = None


def _get_nc():
    global _NC
    if _NC is None:
        _NC = build_nc()
        _NC.finalize()
    return _NC


def make_in_maps(x, w_qkv, w_proj, b_proj):
    x2 = np.ascontiguousarray(x.reshape(C, N)).astype(ml_dtypes.bfloat16)
    in_maps = []
    for c in range(NCORES):
        sl = slice(8 * c, 8 * c + 8)
        wq = np.ascontiguousarray(w_qkv[sl, :].T).astype(ml_dtypes.bfloat16)
        wk = np.ascontiguousarray(w_qkv[64 + 8 * c : 64 + 8 * c + 8, :].T).astype(
            ml_dtypes.bfloat16
        )
        wv = np.ascontiguousarray(w_qkv[128 + 8 * c : 128 + 8 * c + 8, :].T).astype(
            ml_dtypes.bfloat16
        )
        wp = np.concatenate(
            [w_proj[:, sl].T, (b_proj / NCORES)[None, :]], axis=0
        ).astype(np.float32)
        in_maps.append(
            {"x2": x2, "wq": wq, "wk": wk, "wv": wv, "wp": np.ascontiguousarray(wp)}
        )
    return in_maps


def run(x, w_qkv, w_proj, b_proj, trace=False, **kw):
    nc = _get_nc()
    in_maps = make_in_maps(x, w_qkv, w_proj, b_proj)
    res = run_bass_kernel_spmd(
        nc, in_maps, core_ids=list(range(NCORES)), trace=trace, **kw
    )
    y = np.zeros((N, C), np.float32)
    for r in res.results:
        y += r["y"]
    return y.reshape(1, 12, 12, 12, C), res


def kernel(x, w_qkv, w_proj, b_proj):
    out, _ = run(
        np.asarray(x), np.asarray(w_qkv), np.asarray(w_proj), np.asarray(b_proj)
    )
    return out



# revision 2
# speedup vs baseline: 1.6593x; 1.6593x over previous
"""Trainium2 Bass kernel for nn_Attention (B=1, C=64, 12x12x12 spatial, 32 heads, head_dim=2).

Sharding: 32 heads over 8 cores (4 heads/core), tensor-parallel: per-core
partial output summed on host with bias/8 folded per core.

v2 redesign vs baseline (235us):
- Key chunks of 64 so each [128, 864] score tile stacks TWO heads (kn=64
  rows each) -> exp instructions always process full 128 partitions.
- 4-way PE tile concurrency: S-matmuls at positions (0,0)/(32,64)/(64,0)/
  (96,64), U-matmuls at (0,0)/(64,32)/(0,64)/(64,96), emitted back-to-back
  with no interleaved PE work so the 16x 32x32 sub-array concurrency engages.
- Hybrid exp: ScalarE ACT exp for tile A (+ every 8th tile B), DVE
  Schraudolph for the rest: tensor_scalar computes A16*s + B16 in f32 and
  converts to int16 whose bits ARE the bf16 exp approximation (softmax
  normalization cancels most of the ~6% elementwise error; validated
  numerically at ~0.009 output rel err if used for ALL elements).
- Divide: reciprocal_approx_fast (1 custom DVE op, ~18-bit) straight off the
  PSUM U/Z tile instead of the 8-cyc/elem exact reciprocal; 1/Z rows are
  dup'd to the U rows by two SBUF->SBUF DMAs, one DVE multiply finishes O.
"""

import numpy as np
import ml_dtypes

import concourse.bass as bass
import concourse.bacc as bacc
import concourse.mybir as mybir
from concourse import tile
from concourse.bass_utils import run_bass_kernel_spmd

C = 64
N = 1728
NCORES = 8
HLOC = 4
SCALE = float(2.0 ** -0.5)

KN = 64
NKC = N // KN            # 27 key chunks
QTS = [(0, 864), (864, 864)]
SUBS = [(0, 512), (512, 352)]
QKCH = [(0, 512), (512, 512), (1024, 512), (1536, 192)]

LOG2E = 1.4426950408889634
A16 = SCALE * 128.0 * LOG2E
B16 = 127.0 * 128.0

F32 = mybir.dt.float32
BF16 = mybir.dt.bfloat16
I16 = mybir.dt.int16
EXP = mybir.ActivationFunctionType.Exp
MUL = mybir.AluOpType.mult
ADD = mybir.AluOpType.add
DIV = mybir.AluOpType.divide

# (tile, half, colband) per head for S and U matmuls
# S: head h scores land in (psA|psB)[half:half+64] via tile_position (32h, half)
S_POS = [(0, 0), (0, 64), (1, 0), (1, 64)]   # h -> (tileAB, half)
# U: head h -> pu[cb:cb+3] via tile_position (half, cb), weight vp[half][g]
U_POS = [(0, 0), (64, 32), (0, 64), (64, 96)]  # h -> (half, colband)


def build_nc(debug=False):
    nc = bacc.Bacc(None)

    x2 = nc.declare_dram_parameter("x2", [C, N], BF16, isOutput=False)
    w24 = nc.declare_dram_parameter("w24", [C, 3 * 2 * HLOC], BF16, isOutput=False)
    wp = nc.declare_dram_parameter("wp", [2 * HLOC + 1, C], F32, isOutput=False)
    y = nc.declare_dram_parameter("y", [N, C], F32, isOutput=True)
    if debug:
        d_qT = nc.declare_dram_parameter("d_qT", [128, N], BF16, isOutput=True)
        d_kT = nc.declare_dram_parameter("d_kT", [128, N], BF16, isOutput=True)
        d_vp = nc.declare_dram_parameter("d_vp", [128, NKC * 6], BF16, isOutput=True)
        d_ot = nc.declare_dram_parameter("d_ot", [9, N], F32, isOutput=True)
        d_u = nc.declare_dram_parameter("d_u", [128, 1024], F32, isOutput=True)

    with tile.TileContext(nc) as tc:
        with (
            tc.tile_pool(name="const", bufs=1) as cpool,
            tc.tile_pool(name="epool", bufs=4) as epool,
            tc.tile_pool(name="dpool", bufs=2) as dpool,
            tc.tile_pool(name="ps", bufs=3, space=bass.MemorySpace.PSUM) as ps,
            tc.tile_pool(name="psu", bufs=1, space=bass.MemorySpace.PSUM) as psu,
        ):
            x_sb = cpool.tile([C, N], BF16, name="x_sb")
            w24_sb = cpool.tile([C, 24], BF16, name="w24_sb")
            wp_sb = cpool.tile([2 * HLOC + 1, C], F32, name="wp_sb")
            qT = cpool.tile([128, N], BF16, name="qT")
            kT = cpool.tile([128, N], BF16, name="kT")
            vp = cpool.tile([128, NKC * 2 * 3], BF16, name="vp")
            ot = cpool.tile([2 * HLOC + 1, N], F32, name="ot")
            yb0 = cpool.tile([108, 512], F32, name="yb0")
            yb1 = cpool.tile([108, 512], F32, name="yb1")
            ybs = [yb0, yb1]

            nc.sync.dma_start(out=x_sb[:], in_=x2[:])
            nc.sync.dma_start(out=w24_sb[:], in_=w24[:])
            nc.sync.dma_start(out=wp_sb[:], in_=wp[:])

            vp_v = vp[:].rearrange("p (kc g c) -> p kc g c", g=2, c=3)
            # rows 0..7 are overwritten by the divide unpack; row 8 stays 1.0
            nc.gpsimd.memset(ot[:, :], 1.0)
            nc.gpsimd.memset(vp_v[:, :, :, 2:3], 1.0)

            # ---- qkv projections, col-tiled 4 heads at once ----
            # q/k: per chunk: out[32h:32h+2, :] = w24[:, 2h:2h+2].T @ x
            for dst, wofs, eng in ((qT, 0, 0), (kT, 8, 1)):
                for ci, (co, cn) in enumerate(QKCH):
                    pq = ps.tile([128, 512], F32, tag="s", name="pq")
                    for h in range(HLOC):
                        nc.tensor.matmul(
                            pq[32 * h : 32 * h + 2, :cn],
                            w24_sb[:, wofs + 2 * h : wofs + 2 * h + 2],
                            x_sb[:, co : co + cn],
                            start=True, stop=True,
                            tile_position=(0, 32 * h),
                        )
                    # alternate copy engine to split the load
                    if (ci + eng) % 2 == 0:
                        nc.vector.tensor_copy(dst[:, co : co + cn], pq[:, :cn])
                    else:
                        nc.scalar.copy(dst[:, co : co + cn], pq[:, :cn])

            # ---- V': x chunks as weights; duplicate into both halves ----
            psv = ps.tile([128, NKC * 8], F32, tag="s", name="psv")
            for kc in range(NKC):
                for half in (0, 64):
                    nc.tensor.matmul(
                        psv[half : half + 64, 8 * kc : 8 * kc + 8],
                        x_sb[:, KN * kc : KN * kc + KN],
                        w24_sb[:, 16:24],
                        start=True, stop=True,
                        tile_position=(0, half),
                    )
            # psv cols per kc: (h, d) h-major. lo half keeps h0,h2; hi h1,h3
            psv_lo = psv[0:64, :].rearrange("p (kc g d) -> p kc g d", g=2, d=4)
            psv_hi = psv[64:128, :].rearrange("p (kc g d) -> p kc g d", g=2, d=4)
            nc.vector.tensor_copy(vp_v[0:64, :, :, 0:2], psv_lo[:, :, :, 0:2])
            nc.vector.tensor_copy(vp_v[64:128, :, :, 0:2], psv_hi[:, :, :, 2:4])

            # ---- main attention loops ----
            def emit_S(qo, qn, kc):
                pa = ps.tile([128, 1024], F32, tag="s", name="pa")
                pb = ps.tile([128, 1024], F32, tag="s", name="pb")
                tiles = (pa, pb)
                for (o, n_) in SUBS:
                    if o >= qn:
                        continue
                    n_ = min(n_, qn - o)
                    for h in range(HLOC):
                        ab, half = S_POS[h]
                        nc.tensor.matmul(
                            tiles[ab][half : half + KN, o : o + n_],
                            kT[32 * h : 32 * h + 2, KN * kc : KN * kc + KN],
                            qT[32 * h : 32 * h + 2, qo + o : qo + o + n_],
                            start=True, stop=True,
                            tile_position=(32 * h, half),
                        )
                return pa, pb

            def emit_exp(qn, kc, pa, pb, b_on_dve):
                ea = epool.tile([128, 1024], BF16, tag="e", name="ea")
                eb = epool.tile([128, 1024], BF16, tag="e", name="eb")
                nc.scalar.activation(ea[:, :qn], pa[:, :qn], EXP, scale=SCALE)
                if b_on_dve:
                    nc.vector.tensor_scalar(
                        eb[:, :qn].bitcast(I16), pb[:, :qn], A16, B16, MUL, ADD,
                    )
                else:
                    nc.scalar.activation(eb[:, :qn], pb[:, :qn], EXP, scale=SCALE)
                return ea, eb

            def emit_U(qn, kc, ea, eb, pu):
                es_ = (ea, eb)
                for (o, n_) in SUBS:
                    if o >= qn:
                        continue
                    n_ = min(n_, qn - o)
                    for h in range(HLOC):
                        ab, half = S_POS[h]
                        _, cb = U_POS[h]
                        nc.tensor.matmul(
                            pu[cb : cb + 3, o : o + n_],
                            vp_v[half : half + 64, kc, h // 2, :],
                            es_[ab][half : half + 64, o : o + n_],
                            start=(kc == 0), stop=(kc == NKC - 1),
                            tile_position=(half, cb),
                        )

            def emit_divide(qo, qn, pu):
                # zz = 1/pu elementwise (~18-bit approx; only rows 32h+2 = 1/Z
                # matter -- U rows' 1/x is overwritten by the dup DMAs below or
                # never read). Z > 0 always so no undefined edge cases there.
                zz = dpool.tile([128, 1024], F32, tag="zz", name="zz")
                zz_v = zz[:, :qn].rearrange("(h g) n -> h g n", g=32)
                nc.vector.reciprocal_approx_fast(zz[:, :qn], pu[:, :qn])
                nc.sync.dma_start(out=zz_v[:, 0, :], in_=zz_v[:, 2, :])
                nc.gpsimd.dma_start(out=zz_v[:, 1, :], in_=zz_v[:, 2, :])
                osp = dpool.tile([128, 1024], F32, tag="osp", name="osp")
                nc.vector.tensor_mul(osp[:, :qn], pu[:, :qn], zz[:, :qn])
                ov = osp[:, :qn].rearrange("(h g) n -> h g n", g=32)
                ot_v = ot[0 : 2 * HLOC, qo : qo + qn].rearrange("(h d) n -> h d n", d=2)
                nc.sync.dma_start(out=ot_v[:, 0, :], in_=ov[:, 0, :])
                nc.gpsimd.dma_start(out=ot_v[:, 1, :], in_=ov[:, 1, :])
                return zz

            def emit_proj(tstart, nch):
                """nch proj chunks -> small psum tile -> immediate copy to yb."""
                py = ps.tile([108, 512], F32, tag="s", name="py")
                for t8 in range(nch):
                    t = tstart + t8
                    nc.tensor.matmul(
                        py[:108, 64 * t8 : 64 * t8 + 64],
                        ot[:, 108 * t : 108 * t + 108],
                        wp_sb[:],
                        start=True, stop=True,
                    )
                yb = ybs[tstart // 8]
                o8 = (tstart % 8) * 64
                nc.vector.tensor_copy(yb[:, o8 : o8 + 64 * nch], py[:108, : 64 * nch])

            def emit_proj_out(half):
                yb = ybs[half]
                yv = y[864 * half : 864 * (half + 1), :].rearrange(
                    "(t i) c -> i t c", i=108
                )
                nc.sync.dma_start(out=yv, in_=yb[:].rearrange("p (t c) -> p t c", c=64))

            def qt_loop(qo, qn, hooks):
                pu = psu.tile([128, 1024], F32, tag="u", name="pu")
                pend = None
                for kc in range(NKC):
                    pa, pb = emit_S(qo, qn, kc)
                    if pend is not None:
                        emit_U(qn, pend[0], pend[1], pend[2], pu)
                    hk = hooks.get(kc)
                    if hk:
                        hk()
                    ea, eb = emit_exp(qn, kc, pa, pb, b_on_dve=(kc % 8 != 0))
                    pend = (kc, ea, eb)
                emit_U(qn, pend[0], pend[1], pend[2], pu)
                return pu

            pu0 = qt_loop(0, 864, {})
            zz0 = emit_divide(0, 864, pu0)

            # qt1: overlap qt0's proj with the attention loop
            hooks1 = {
                5: lambda: emit_proj(0, 2),
                7: lambda: emit_proj(2, 2),
                9: lambda: emit_proj(4, 2),
                11: lambda: emit_proj(6, 2),
                13: lambda: emit_proj_out(0),
            }
            pu1 = qt_loop(864, 864, hooks1)
            emit_divide(864, 864, pu1)
            for ts_ in (8, 10, 12, 14):
                emit_proj(ts_, 2)
            emit_proj_out(1)

            if debug:
                nc.sync.dma_start(out=d_qT[:], in_=qT[:])
                nc.sync.dma_start(out=d_kT[:], in_=kT[:])
                nc.sync.dma_start(out=d_vp[:], in_=vp[:])
                nc.sync.dma_start(out=d_ot[:], in_=ot[:])
                nc.sync.dma_start(out=d_u[:], in_=zz0[:])

    return nc


_NC = None


def _get_nc():
    global _NC
    if _NC is None:
        _NC = build_nc()
        _NC.finalize()
    return _NC


def make_in_maps(x, w_qkv, w_proj, b_proj):
    x2 = np.ascontiguousarray(x.reshape(C, N)).astype(ml_dtypes.bfloat16)
    in_maps = []
    for c in range(NCORES):
        sl = slice(8 * c, 8 * c + 8)
        w24 = np.concatenate(
            [
                w_qkv[sl, :].T,
                w_qkv[64 + 8 * c : 64 + 8 * c + 8, :].T,
                w_qkv[128 + 8 * c : 128 + 8 * c + 8, :].T,
            ],
            axis=1,
        ).astype(ml_dtypes.bfloat16)
        wp = np.concatenate(
            [w_proj[:, sl].T, (b_proj / NCORES)[None, :]], axis=0
        ).astype(np.float32)
        in_maps.append(
            {
                "x2": x2,
                "w24": np.ascontiguousarray(w24),
                "wp": np.ascontiguousarray(wp),
            }
        )
    return in_maps


def run(x, w_qkv, w_proj, b_proj, trace=False, **kw):
    nc = _get_nc()
    in_maps = make_in_maps(x, w_qkv, w_proj, b_proj)
    res = run_bass_kernel_spmd(
        nc, in_maps, core_ids=list(range(NCORES)), trace=trace, **kw
    )
    y = np.zeros((N, C), np.float32)
    for r in res.results:
        y += r["y"]
    return y.reshape(1, 12, 12, 12, C), res


def kernel(x, w_qkv, w_proj, b_proj):
    out, _ = run(
        np.asarray(x), np.asarray(w_qkv), np.asarray(w_proj), np.asarray(b_proj)
    )
    return out


# revision 4
# speedup vs baseline: 1.8497x; 1.1147x over previous
"""Trainium2 Bass kernel for nn_Attention (B=1, C=64, 12x12x12 spatial, 32 heads, head_dim=2).

Sharding: 32 heads over 8 cores (4 heads/core), tensor-parallel: per-core
partial output summed on host with bias/8 folded per core.

v2 redesign vs baseline (235us):
- Key chunks of 64 so each [128, 864] score tile stacks TWO heads (kn=64
  rows each) -> exp instructions always process full 128 partitions.
- 4-way PE tile concurrency: S-matmuls at positions (0,0)/(32,64)/(64,0)/
  (96,64), U-matmuls at (0,0)/(64,32)/(0,64)/(64,96), emitted back-to-back
  with no interleaved PE work so the 16x 32x32 sub-array concurrency engages.
- Hybrid exp: ScalarE ACT exp for tile A (+ every 8th tile B), DVE
  Schraudolph for the rest: tensor_scalar computes A16*s + B16 in f32 and
  converts to int16 whose bits ARE the bf16 exp approximation (softmax
  normalization cancels most of the ~6% elementwise error; validated
  numerically at ~0.009 output rel err if used for ALL elements).
- Divide: reciprocal_approx_fast (1 custom DVE op, ~18-bit) straight off the
  PSUM U/Z tile instead of the 8-cyc/elem exact reciprocal; 1/Z rows are
  dup'd to the U rows by two SBUF->SBUF DMAs, one DVE multiply finishes O.
"""

import numpy as np
import ml_dtypes

import concourse.bass as bass
import concourse.bacc as bacc
import concourse.mybir as mybir
from concourse import tile
from concourse.bass_utils import run_bass_kernel_spmd

C = 64
N = 1728
NCORES = 8
HLOC = 4
SCALE = float(2.0 ** -0.5)

KN = 64
NKC = N // KN            # 27 key chunks
QTS = [(0, 864), (864, 864)]
SUBS = [(0, 512), (512, 352)]
QKCH = [(0, 512), (512, 512), (1024, 512), (1536, 192)]

LOG2E = 1.4426950408889634
A16 = SCALE * 128.0 * LOG2E
B16 = 127.0 * 128.0

F32 = mybir.dt.float32
BF16 = mybir.dt.bfloat16
I16 = mybir.dt.int16
EXP = mybir.ActivationFunctionType.Exp
MUL = mybir.AluOpType.mult
ADD = mybir.AluOpType.add
DIV = mybir.AluOpType.divide

# (tile, half, colband) per head for S and U matmuls
# S: head h scores land in (psA|psB)[half:half+64] via tile_position (32h, half)
S_POS = [(0, 0), (0, 64), (1, 0), (1, 64)]   # h -> (tileAB, half)
# U: head h -> pu[cb:cb+3] via tile_position (half, cb), weight vp[half][g]
U_POS = [(0, 0), (64, 32), (0, 64), (64, 96)]  # h -> (half, colband)


def build_nc(debug=False):
    nc = bacc.Bacc(None)

    x2 = nc.declare_dram_parameter("x2", [C, N], BF16, isOutput=False)
    w24 = nc.declare_dram_parameter("w24", [C, 3 * 2 * HLOC], BF16, isOutput=False)
    wp = nc.declare_dram_parameter("wp", [2 * HLOC + 1, C], F32, isOutput=False)
    y = nc.declare_dram_parameter("y", [N, C], F32, isOutput=True)
    if debug:
        d_qT = nc.declare_dram_parameter("d_qT", [128, N], BF16, isOutput=True)
        d_kT = nc.declare_dram_parameter("d_kT", [128, N], BF16, isOutput=True)
        d_vp = nc.declare_dram_parameter("d_vp", [128, NKC * 6], BF16, isOutput=True)
        d_ot = nc.declare_dram_parameter("d_ot", [9, N], F32, isOutput=True)
        d_u = nc.declare_dram_parameter("d_u", [128, 1024], F32, isOutput=True)

    with tile.TileContext(nc) as tc:
        with (
            tc.tile_pool(name="const", bufs=1) as cpool,
            tc.tile_pool(name="epool", bufs=4) as epool,
            tc.tile_pool(name="dpool", bufs=2) as dpool,
            tc.tile_pool(name="ps", bufs=3, space=bass.MemorySpace.PSUM) as ps,
            tc.tile_pool(name="psu", bufs=1, space=bass.MemorySpace.PSUM) as psu,
        ):
            x_sb = cpool.tile([C, N], BF16, name="x_sb")
            w24_sb = cpool.tile([C, 24], BF16, name="w24_sb")
            wp_sb = cpool.tile([2 * HLOC + 1, C], F32, name="wp_sb")
            qT = cpool.tile([128, N], BF16, name="qT")
            kT = cpool.tile([128, N], BF16, name="kT")
            vp = cpool.tile([128, NKC * 2 * 3], BF16, name="vp")
            ot = cpool.tile([2 * HLOC + 1, N], F32, name="ot")
            yb0 = cpool.tile([108, 512], F32, name="yb0")
            yb1 = cpool.tile([108, 512], F32, name="yb1")
            ybs = [yb0, yb1]

            nc.sync.dma_start(out=x_sb[:], in_=x2[:])
            nc.sync.dma_start(out=w24_sb[:], in_=w24[:])
            nc.sync.dma_start(out=wp_sb[:], in_=wp[:])

            vp_v = vp[:].rearrange("p (kc g c) -> p kc g c", g=2, c=3)
            # rows 0..7 are overwritten by the divide unpack; row 8 stays 1.0
            nc.gpsimd.memset(ot[:, :], 1.0)
            nc.gpsimd.memset(vp_v[:, :, :, 2:3], 1.0)

            # ---- qkv projections, col-tiled 4 heads at once ----
            # q/k: per chunk: out[32h:32h+2, :] = w24[:, 2h:2h+2].T @ x
            for dst, wofs, eng in ((qT, 0, 0), (kT, 8, 1)):
                for ci, (co, cn) in enumerate(QKCH):
                    pq = ps.tile([128, 512], F32, tag="s", name="pq")
                    for h in range(HLOC):
                        nc.tensor.matmul(
                            pq[32 * h : 32 * h + 2, :cn],
                            w24_sb[:, wofs + 2 * h : wofs + 2 * h + 2],
                            x_sb[:, co : co + cn],
                            start=True, stop=True,
                            tile_position=(0, 32 * h),
                        )
                    # alternate copy engine to split the load
                    if (ci + eng) % 2 == 0:
                        nc.vector.tensor_copy(dst[:, co : co + cn], pq[:, :cn])
                    else:
                        nc.scalar.copy(dst[:, co : co + cn], pq[:, :cn])

            # ---- V': x chunks as weights; duplicate into both halves ----
            psv = ps.tile([128, NKC * 8], F32, tag="s", name="psv")
            for kc in range(NKC):
                for half in (0, 64):
                    nc.tensor.matmul(
                        psv[half : half + 64, 8 * kc : 8 * kc + 8],
                        x_sb[:, KN * kc : KN * kc + KN],
                        w24_sb[:, 16:24],
                        start=True, stop=True,
                        tile_position=(0, half),
                    )
            # psv cols per kc: (h, d) h-major. lo half keeps h0,h2; hi h1,h3
            psv_lo = psv[0:64, :].rearrange("p (kc g d) -> p kc g d", g=2, d=4)
            psv_hi = psv[64:128, :].rearrange("p (kc g d) -> p kc g d", g=2, d=4)
            nc.vector.tensor_copy(vp_v[0:64, :, :, 0:2], psv_lo[:, :, :, 0:2])
            nc.vector.tensor_copy(vp_v[64:128, :, :, 0:2], psv_hi[:, :, :, 2:4])

            # ---- main attention loops ----
            def emit_S(qo, qn, kc):
                pa = ps.tile([128, 1024], F32, tag="s", name="pa")
                pb = ps.tile([128, 1024], F32, tag="s", name="pb")
                tiles = (pa, pb)
                for (o, n_) in SUBS:
                    if o >= qn:
                        continue
                    n_ = min(n_, qn - o)
                    for h in range(HLOC):
                        ab, half = S_POS[h]
                        nc.tensor.matmul(
                            tiles[ab][half : half + KN, o : o + n_],
                            kT[32 * h : 32 * h + 2, KN * kc : KN * kc + KN],
                            qT[32 * h : 32 * h + 2, qo + o : qo + o + n_],
                            start=True, stop=True,
                            tile_position=(32 * h, half),
                        )
                return pa, pb

            def emit_exp(qn, kc, pa, pb, b_on_dve):
                ea = epool.tile([128, 1024], BF16, tag="e", name="ea")
                eb = epool.tile([128, 1024], BF16, tag="e", name="eb")
                nc.scalar.activation(ea[:, :qn], pa[:, :qn], EXP, scale=SCALE)
                if b_on_dve:
                    nc.vector.tensor_scalar(
                        eb[:, :qn].bitcast(I16), pb[:, :qn], A16, B16, MUL, ADD,
                    )
                else:
                    nc.scalar.activation(eb[:, :qn], pb[:, :qn], EXP, scale=SCALE)
                return ea, eb

            def emit_U(qn, kc, ea, eb, pu):
                es_ = (ea, eb)
                for (o, n_) in SUBS:
                    if o >= qn:
                        continue
                    n_ = min(n_, qn - o)
                    for h in range(HLOC):
                        ab, half = S_POS[h]
                        _, cb = U_POS[h]
                        nc.tensor.matmul(
                            pu[cb : cb + 3, o : o + n_],
                            vp_v[half : half + 64, kc, h // 2, :],
                            es_[ab][half : half + 64, o : o + n_],
                            start=(kc == 0), stop=(kc == NKC - 1),
                            tile_position=(half, cb),
                        )

            def emit_divide(qo, qn, pu):
                # zz = 1/pu elementwise (~18-bit approx; only rows 32h+2 = 1/Z
                # matter -- U rows' 1/x is overwritten by the dup DMAs below or
                # never read). Z > 0 always so no undefined edge cases there.
                zz = dpool.tile([128, 1024], F32, tag="zz", name="zz")
                zz_v = zz[:, :qn].rearrange("(h g) n -> h g n", g=32)
                nc.vector.reciprocal_approx_fast(zz[:, :qn], pu[:, :qn])
                nc.sync.dma_start(out=zz_v[:, 0, :], in_=zz_v[:, 2, :])
                nc.gpsimd.dma_start(out=zz_v[:, 1, :], in_=zz_v[:, 2, :])
                osp = dpool.tile([128, 1024], F32, tag="osp", name="osp")
                nc.vector.tensor_mul(osp[:, :qn], pu[:, :qn], zz[:, :qn])
                ov = osp[:, :qn].rearrange("(h g) n -> h g n", g=32)
                ot_v = ot[0 : 2 * HLOC, qo : qo + qn].rearrange("(h d) n -> h d n", d=2)
                nc.sync.dma_start(out=ot_v[:, 0, :], in_=ov[:, 0, :])
                nc.gpsimd.dma_start(out=ot_v[:, 1, :], in_=ov[:, 1, :])
                return zz

            def emit_proj(tstart, nch):
                """nch proj chunks -> small psum tile -> immediate copy to yb."""
                py = ps.tile([108, 512], F32, tag="s", name="py")
                for t8 in range(nch):
                    t = tstart + t8
                    nc.tensor.matmul(
                        py[:108, 64 * t8 : 64 * t8 + 64],
                        ot[:, 108 * t : 108 * t + 108],
                        wp_sb[:],
                        start=True, stop=True,
                    )
                yb = ybs[tstart // 8]
                o8 = (tstart % 8) * 64
                nc.vector.tensor_copy(yb[:, o8 : o8 + 64 * nch], py[:108, : 64 * nch])

            def emit_proj_out(half):
                yb = ybs[half]
                yv = y[864 * half : 864 * (half + 1), :].rearrange(
                    "(t i) c -> i t c", i=108
                )
                nc.sync.dma_start(out=yv, in_=yb[:].rearrange("p (t c) -> p t c", c=64))

            def qt_loop(qo, qn, hooks):
                pu = psu.tile([128, 1024], F32, tag="u", name="pu")
                pend = None
                for kc in range(NKC):
                    pa, pb = emit_S(qo, qn, kc)
                    if pend is not None:
                        emit_U(qn, pend[0], pend[1], pend[2], pu)
                    hk = hooks.get(kc)
                    if hk:
                        hk()
                    ea, eb = emit_exp(qn, kc, pa, pb, b_on_dve=(kc % 8 != 0))
                    pend = (kc, ea, eb)
                emit_U(qn, pend[0], pend[1], pend[2], pu)
                return pu

            pu0 = qt_loop(0, 864, {})
            zz0 = emit_divide(0, 864, pu0)

            # qt1: overlap qt0's proj with the attention loop
            hooks1 = {
                5: lambda: emit_proj(0, 2),
                7: lambda: emit_proj(2, 2),
                9: lambda: emit_proj(4, 2),
                11: lambda: emit_proj(6, 2),
                13: lambda: emit_proj_out(0),
            }
            pu1 = qt_loop(864, 864, hooks1)
            emit_divide(864, 864, pu1)
            for ts_ in (8, 10, 12, 14):
                emit_proj(ts_, 2)
            emit_proj_out(1)

            if debug:
                nc.sync.dma_start(out=d_qT[:], in_=qT[:])
                nc.sync.dma_start(out=d_kT[:], in_=kT[:])
                nc.sync.dma_start(out=d_vp[:], in_=vp[:])
                nc.sync.dma_start(out=d_ot[:], in_=ot[:])
                nc.sync.dma_start(out=d_u[:], in_=zz0[:])

    return nc


_NC = None


def _get_nc():
    global _NC
    if _NC is None:
        _NC = build_nc()
        _NC.finalize()
    return _NC


def make_in_maps(x, w_qkv, w_proj, b_proj):
    x2 = np.ascontiguousarray(x.reshape(C, N)).astype(ml_dtypes.bfloat16)
    in_maps = []
    for c in range(NCORES):
        sl = slice(8 * c, 8 * c + 8)
        w24 = np.concatenate(
            [
                w_qkv[sl, :].T,
                w_qkv[64 + 8 * c : 64 + 8 * c + 8, :].T,
                w_qkv[128 + 8 * c : 128 + 8 * c + 8, :].T,
            ],
            axis=1,
        ).astype(ml_dtypes.bfloat16)
        wp = np.concatenate(
            [w_proj[:, sl].T, (b_proj / NCORES)[None, :]], axis=0
        ).astype(np.float32)
        in_maps.append(
            {
                "x2": x2,
                "w24": np.ascontiguousarray(w24),
                "wp": np.ascontiguousarray(wp),
            }
        )
    return in_maps


def run(x, w_qkv, w_proj, b_proj, trace=False, **kw):
    nc = _get_nc()
    in_maps = make_in_maps(x, w_qkv, w_proj, b_proj)
    res = run_bass_kernel_spmd(
        nc, in_maps, core_ids=list(range(NCORES)), trace=trace, **kw
    )
    y = np.zeros((N, C), np.float32)
    for r in res.results:
        y += r["y"]
    return y.reshape(1, 12, 12, 12, C), res


def kernel(x, w_qkv, w_proj, b_proj):
    out, _ = run(
        np.asarray(x), np.asarray(w_qkv), np.asarray(w_proj), np.asarray(b_proj)
    )
    return out


# revision 5
# speedup vs baseline: 1.9465x; 1.0523x over previous
"""Trainium2 Bass kernel for nn_Attention (B=1, C=64, 12x12x12 spatial, 32 heads, head_dim=2).

Sharding: 32 heads over 8 cores (4 heads/core), tensor-parallel: per-core
partial output summed on host with bias/8 folded per core.

v4 vs v2 (141us): key chunks processed in PAIRS (kn=128), single-head score tiles.
- S-matmuls run 8-way concurrent: (32h, 64*parity) for 4 heads x 2 x 64-key
  chunks = all 16 PE sub-arrays; each pair-iter's S wall is ~qn columns.
- U-matmuls contract 128 keys per pass (halves U column-streams), 4-way
  col-tiled at (0, 32h).
- Two heads share one [128, 1024] f32 score tile (h at col offset qn*(h%2))
  so each exp instruction runs at FD=2*qn (amortizes engine overhead).
- Hybrid exp: ScalarE ACT exp for tile AB (+ every 4th CD), DVE Schraudolph
  (tensor_scalar f32->i16; bits are the bf16 exp approx) for the rest.
- Query tiles of 512 (PSUM bank limit): tiles AB/CD are 2 banks each,
  pool of 3 + pu 1 bank = 7 of 8 banks.
"""

import numpy as np
import ml_dtypes

import concourse.bass as bass
import concourse.bacc as bacc
import concourse.mybir as mybir
from concourse import tile
from concourse.bass_utils import run_bass_kernel_spmd

C = 64
N = 1728
NCORES = 8
HLOC = 4
SCALE = float(2.0 ** -0.5)

QTS = [(0, 512), (512, 512), (1024, 512), (1536, 192)]
PAIRS = [(128 * p, 128) for p in range(13)] + [(1664, 64)]
NPR = len(PAIRS)
QKCH = [(0, 512), (512, 512), (1024, 512), (1536, 192)]

LOG2E = 1.4426950408889634
A16 = SCALE * 128.0 * LOG2E
B16 = 127.0 * 128.0

F32 = mybir.dt.float32
BF16 = mybir.dt.bfloat16
I16 = mybir.dt.int16
EXP = mybir.ActivationFunctionType.Exp
MUL = mybir.AluOpType.mult
ADD = mybir.AluOpType.add


def build_nc(debug=False):
    nc = bacc.Bacc(None)

    x2 = nc.declare_dram_parameter("x2", [C, N], BF16, isOutput=False)
    w24 = nc.declare_dram_parameter("w24", [C, 3 * 2 * HLOC], BF16, isOutput=False)
    wp = nc.declare_dram_parameter("wp", [2 * HLOC + 1, C], F32, isOutput=False)
    y = nc.declare_dram_parameter("y", [N, C], F32, isOutput=True)
    if debug:
        d_qT = nc.declare_dram_parameter("d_qT", [128, N], BF16, isOutput=True)
        d_kT = nc.declare_dram_parameter("d_kT", [128, N], BF16, isOutput=True)
        d_vp = nc.declare_dram_parameter("d_vp", [128, NPR * 12], BF16, isOutput=True)
        d_ot = nc.declare_dram_parameter("d_ot", [9, N], F32, isOutput=True)
        d_u = nc.declare_dram_parameter("d_u", [128, 512], F32, isOutput=True)

    with tile.TileContext(nc) as tc:
        with (
            tc.tile_pool(name="const", bufs=1) as cpool,
            tc.tile_pool(name="epool", bufs=8) as epool,
            tc.tile_pool(name="dpool", bufs=2) as dpool,
            tc.tile_pool(name="ps", bufs=6, space=bass.MemorySpace.PSUM) as ps,
            tc.tile_pool(name="psu", bufs=1, space=bass.MemorySpace.PSUM) as psu,
        ):
            x_sb = cpool.tile([C, N], BF16, name="x_sb")
            w24_sb = cpool.tile([C, 24], BF16, name="w24_sb")
            wp_sb = cpool.tile([2 * HLOC + 1, C], F32, name="wp_sb")
            qT = cpool.tile([128, N], BF16, name="qT")
            kT = cpool.tile([128, N], BF16, name="kT")
            vp = cpool.tile([128, NPR * HLOC * 3], BF16, name="vp")
            ot = cpool.tile([2 * HLOC + 1, N], F32, name="ot")
            yb0 = cpool.tile([108, 512], F32, name="yb0")
            yb1 = cpool.tile([108, 512], F32, name="yb1")
            ybs = [yb0, yb1]

            nc.sync.dma_start(out=x_sb[:], in_=x2[:])
            nc.sync.dma_start(out=w24_sb[:], in_=w24[:])
            nc.sync.dma_start(out=wp_sb[:], in_=wp[:])

            vp_v = vp[:].rearrange("p (pr h c) -> p pr h c", h=HLOC, c=3)
            # ot rows 0..7 are overwritten by the divide unpack; row 8 stays 1.0
            nc.gpsimd.memset(ot[:, :], 1.0)
            nc.gpsimd.memset(vp_v[:, :, :, 2:3], 1.0)

            # ---- qkv projections, col-tiled 4 heads at once ----
            for dst, wofs, eng in ((qT, 0, 0), (kT, 8, 1)):
                for ci, (co, cn) in enumerate(QKCH):
                    pq = ps.tile([128, 512], F32, tag="s", name="pq")
                    for h in range(HLOC):
                        nc.tensor.matmul(
                            pq[32 * h : 32 * h + 2, :cn],
                            w24_sb[:, wofs + 2 * h : wofs + 2 * h + 2],
                            x_sb[:, co : co + cn],
                            start=True, stop=True,
                            tile_position=(0, 32 * h),
                        )
                    if (ci + eng) % 2 == 0:
                        nc.vector.tensor_copy(dst[:, co : co + cn], pq[:, :cn])
                    else:
                        nc.scalar.copy(dst[:, co : co + cn], pq[:, :cn])

            # ---- V': x key-chunk-pairs as weights -> [128, 8] per pair ----
            psv = ps.tile([128, NPR * 8], F32, tag="s", name="psv")
            for pr, (ko, kn2) in enumerate(PAIRS):
                nc.tensor.matmul(
                    psv[:kn2, 8 * pr : 8 * pr + 8],
                    x_sb[:, ko : ko + kn2],
                    w24_sb[:, 16:24],
                    start=True, stop=True,
                )
            psv_v = psv[:].rearrange("p (pr h d) -> p pr h d", h=HLOC, d=2)
            nc.vector.tensor_copy(vp_v[:, :, :, 0:2], psv_v[:, :, :, :])

            # ---- main attention loops ----
            def emit_S(qo, qn, pr):
                # one MM per head: weight kT[2, kn2] covers the whole key
                # pair; 4 heads row-tiled at (32h, 0) run concurrently.
                ko, kn2 = PAIRS[pr]
                tiles = [ps.tile([128, 512], F32, tag="s", name=f"sc{h}")
                         for h in range(HLOC)]
                for h in range(HLOC):
                    nc.tensor.matmul(
                        tiles[h][:kn2, :qn],
                        kT[32 * h : 32 * h + 2, ko : ko + kn2],
                        qT[32 * h : 32 * h + 2, qo : qo + qn],
                        start=True, stop=True,
                        tile_position=(32 * h, 0),
                    )
                return tiles

            def emit_exp(qn, sc, n_scalar):
                es = [epool.tile([128, 512], BF16, tag="e", name=f"es{h}")
                      for h in range(HLOC)]
                for h in range(HLOC):
                    if h < n_scalar:
                        nc.scalar.activation(
                            es[h][:, :qn], sc[h][:, :qn], EXP, scale=SCALE)
                    else:
                        nc.vector.tensor_scalar(
                            es[h][:, :qn].bitcast(I16), sc[h][:, :qn],
                            A16, B16, MUL, ADD,
                        )
                return es

            def emit_U(qn, pr, es, pu):
                ko, kn2 = PAIRS[pr]
                for h in range(HLOC):
                    nc.tensor.matmul(
                        pu[32 * h : 32 * h + 3, 0:qn],
                        vp_v[:kn2, pr, h, :],
                        es[h][:kn2, :qn],
                        start=(pr == 0), stop=(pr == NPR - 1),
                        tile_position=(0, 32 * h),
                    )

            def emit_divide(qo, qn, pu):
                zz = dpool.tile([128, 512], F32, tag="zz", name="zz")
                zz_v = zz[:, :qn].rearrange("(h g) n -> h g n", g=32)
                nc.vector.reciprocal_approx_fast(zz[:, :qn], pu[:, :qn])
                nc.sync.dma_start(out=zz_v[:, 0, :], in_=zz_v[:, 2, :])
                nc.gpsimd.dma_start(out=zz_v[:, 1, :], in_=zz_v[:, 2, :])
                osp = dpool.tile([128, 512], F32, tag="osp", name="osp")
                nc.vector.tensor_mul(osp[:, :qn], pu[:, :qn], zz[:, :qn])
                ov = osp[:, :qn].rearrange("(h g) n -> h g n", g=32)
                ot_v = ot[0 : 2 * HLOC, qo : qo + qn].rearrange("(h d) n -> h d n", d=2)
                nc.sync.dma_start(out=ot_v[:, 0, :], in_=ov[:, 0, :])
                nc.gpsimd.dma_start(out=ot_v[:, 1, :], in_=ov[:, 1, :])
                return zz

            def emit_proj(tstart, nch, copy_eng=0):
                py = ps.tile([108, 512], F32, tag="s", name="py")
                for t8 in range(nch):
                    t = tstart + t8
                    nc.tensor.matmul(
                        py[:108, 64 * t8 : 64 * t8 + 64],
                        ot[:, 108 * t : 108 * t + 108],
                        wp_sb[:],
                        start=True, stop=True,
                    )
                yb = ybs[tstart // 8]
                o8 = (tstart % 8) * 64
                if copy_eng == 0:
                    nc.vector.tensor_copy(yb[:, o8 : o8 + 64 * nch], py[:108, : 64 * nch])
                else:
                    nc.scalar.copy(yb[:, o8 : o8 + 64 * nch], py[:108, : 64 * nch])

            def emit_proj_out(half):
                yb = ybs[half]
                yv = y[864 * half : 864 * (half + 1), :].rearrange(
                    "(t i) c -> i t c", i=108
                )
                nc.sync.dma_start(out=yv, in_=yb[:].rearrange("p (t c) -> p t c", c=64))

            def qt_loop(qo, qn, hooks):
                pu = psu.tile([128, 512], F32, tag="u", name="pu")
                pend = None
                for pr in range(NPR):
                    sc = emit_S(qo, qn, pr)
                    if pend is not None:
                        emit_U(qn, pend[0], pend[1], pu)
                    hk = hooks.get(pr)
                    if hk:
                        hk()
                    n_scalar = 3 if pr % 3 == 0 else 2
                    es = emit_exp(qn, sc, n_scalar)
                    pend = (pr, es)
                emit_U(qn, pend[0], pend[1], pu)
                return pu

            for qi, (qo, qn) in enumerate(QTS):
                hooks = {}
                if qi == 1:
                    hooks = {4: lambda: emit_proj(0, 2, 0),
                             7: lambda: emit_proj(2, 2, 1)}
                elif qi == 2:
                    hooks = {4: lambda: emit_proj(4, 2, 0),
                             7: lambda: emit_proj(6, 2, 1),
                             10: lambda: emit_proj_out(0)}
                elif qi == 3:
                    hooks = {4: lambda: emit_proj(8, 2, 0),
                             7: lambda: emit_proj(10, 2, 1)}
                pu = qt_loop(qo, qn, hooks)
                z = emit_divide(qo, qn, pu)
                if qi == 0 and debug:
                    # dump immediately: the dpool buffer is recycled by later qts
                    nc.sync.dma_start(out=d_u[:], in_=z[:])
            emit_proj(12, 2, 0)
            emit_proj(14, 2, 1)
            emit_proj_out(1)

            if debug:
                nc.sync.dma_start(out=d_qT[:], in_=qT[:])
                nc.sync.dma_start(out=d_kT[:], in_=kT[:])
                nc.sync.dma_start(out=d_vp[:], in_=vp[:])
                nc.sync.dma_start(out=d_ot[:], in_=ot[:])

    return nc


_NC = None


def _get_nc():
    global _NC
    if _NC is None:
        _NC = build_nc()
        _NC.finalize()
    return _NC


def make_in_maps(x, w_qkv, w_proj, b_proj):
    x2 = np.ascontiguousarray(x.reshape(C, N)).astype(ml_dtypes.bfloat16)
    in_maps = []
    for c in range(NCORES):
        sl = slice(8 * c, 8 * c + 8)
        w24 = np.concatenate(
            [
                w_qkv[sl, :].T,
                w_qkv[64 + 8 * c : 64 + 8 * c + 8, :].T,
                w_qkv[128 + 8 * c : 128 + 8 * c + 8, :].T,
            ],
            axis=1,
        ).astype(ml_dtypes.bfloat16)
        wp = np.concatenate(
            [w_proj[:, sl].T, (b_proj / NCORES)[None, :]], axis=0
        ).astype(np.float32)
        in_maps.append(
            {
                "x2": x2,
                "w24": np.ascontiguousarray(w24),
                "wp": np.ascontiguousarray(wp),
            }
        )
    return in_maps


def run(x, w_qkv, w_proj, b_proj, trace=False, **kw):
    nc = _get_nc()
    in_maps = make_in_maps(x, w_qkv, w_proj, b_proj)
    res = run_bass_kernel_spmd(
        nc, in_maps, core_ids=list(range(NCORES)), trace=trace, **kw
    )
    y = np.zeros((N, C), np.float32)
    for r in res.results:
        y += r["y"]
    return y.reshape(1, 12, 12, 12, C), res


def kernel(x, w_qkv, w_proj, b_proj):
    out, _ = run(
        np.asarray(x), np.asarray(w_qkv), np.asarray(w_proj), np.asarray(b_proj)
    )
    return out


# revision 6
# speedup vs baseline: 2.0441x; 1.0501x over previous
"""Trainium2 Bass kernel for nn_Attention (B=1, C=64, 12x12x12 spatial, 32 heads, head_dim=2).

Sharding: 32 heads over 8 cores (4 heads/core), tensor-parallel: per-core
partial output summed on host with bias/8 folded per core.

v5 vs v4 (127us): 2+2 exp split, V-prime off the prologue critical path,
interleaved q/k staging, proj 12-13 overlapped into qt3.
- S-matmuls run 8-way concurrent: (32h, 64*parity) for 4 heads x 2 x 64-key
  chunks = all 16 PE sub-arrays; each pair-iter's S wall is ~qn columns.
- U-matmuls contract 128 keys per pass (halves U column-streams), 4-way
  col-tiled at (0, 32h).
- Two heads share one [128, 1024] f32 score tile (h at col offset qn*(h%2))
  so each exp instruction runs at FD=2*qn (amortizes engine overhead).
- Hybrid exp: ScalarE ACT exp for tile AB (+ every 4th CD), DVE Schraudolph
  (tensor_scalar f32->i16; bits are the bf16 exp approx) for the rest.
- Query tiles of 512 (PSUM bank limit): tiles AB/CD are 2 banks each,
  pool of 3 + pu 1 bank = 7 of 8 banks.
"""

import numpy as np
import ml_dtypes

import concourse.bass as bass
import concourse.bacc as bacc
import concourse.mybir as mybir
from concourse import tile
from concourse.bass_utils import run_bass_kernel_spmd

C = 64
N = 1728
NCORES = 8
HLOC = 4
SCALE = float(2.0 ** -0.5)

QTS = [(0, 512), (512, 512), (1024, 512), (1536, 192)]
PAIRS = [(128 * p, 128) for p in range(13)] + [(1664, 64)]
NPR = len(PAIRS)
QKCH = [(0, 512), (512, 512), (1024, 512), (1536, 192)]

LOG2E = 1.4426950408889634
A16 = SCALE * 128.0 * LOG2E
B16 = 127.0 * 128.0

F32 = mybir.dt.float32
BF16 = mybir.dt.bfloat16
I16 = mybir.dt.int16
EXP = mybir.ActivationFunctionType.Exp
MUL = mybir.AluOpType.mult
ADD = mybir.AluOpType.add


def build_nc(debug=False):
    nc = bacc.Bacc(None)

    x2 = nc.declare_dram_parameter("x2", [C, N], BF16, isOutput=False)
    w24 = nc.declare_dram_parameter("w24", [C, 3 * 2 * HLOC], BF16, isOutput=False)
    wp = nc.declare_dram_parameter("wp", [2 * HLOC + 1, C], F32, isOutput=False)
    y = nc.declare_dram_parameter("y", [N, C], F32, isOutput=True)
    if debug:
        d_qT = nc.declare_dram_parameter("d_qT", [128, N], BF16, isOutput=True)
        d_kT = nc.declare_dram_parameter("d_kT", [128, N], BF16, isOutput=True)
        d_vp = nc.declare_dram_parameter("d_vp", [128, NPR * 12], BF16, isOutput=True)
        d_ot = nc.declare_dram_parameter("d_ot", [9, N], F32, isOutput=True)
        d_u = nc.declare_dram_parameter("d_u", [128, 512], F32, isOutput=True)

    with tile.TileContext(nc) as tc:
        with (
            tc.tile_pool(name="const", bufs=1) as cpool,
            tc.tile_pool(name="epool", bufs=8) as epool,
            tc.tile_pool(name="dpool", bufs=2) as dpool,
            tc.tile_pool(name="ps", bufs=6, space=bass.MemorySpace.PSUM) as ps,
            tc.tile_pool(name="psu", bufs=1, space=bass.MemorySpace.PSUM) as psu,
        ):
            x_sb = cpool.tile([C, N], BF16, name="x_sb")
            w24_sb = cpool.tile([C, 24], BF16, name="w24_sb")
            wp_sb = cpool.tile([2 * HLOC + 1, C], F32, name="wp_sb")
            qT = cpool.tile([128, N], BF16, name="qT")
            kT = cpool.tile([128, N], BF16, name="kT")
            vp = cpool.tile([128, NPR * HLOC * 3], BF16, name="vp")
            ot = cpool.tile([2 * HLOC + 1, N], F32, name="ot")
            yb0 = cpool.tile([108, 512], F32, name="yb0")
            yb1 = cpool.tile([108, 512], F32, name="yb1")
            ybs = [yb0, yb1]

            nc.sync.dma_start(out=x_sb[:], in_=x2[:])
            nc.sync.dma_start(out=w24_sb[:], in_=w24[:])
            nc.sync.dma_start(out=wp_sb[:], in_=wp[:])

            vp_v = vp[:].rearrange("p (pr h c) -> p pr h c", h=HLOC, c=3)
            # ot rows 0..7 are overwritten by the divide unpack; row 8 stays 1.0
            nc.gpsimd.memset(ot[:, :], 1.0)
            nc.gpsimd.memset(vp_v[:, :, :, 2:3], 1.0)

            # ---- qkv projections, col-tiled 4 heads at once ----
            for ci, (co, cn) in enumerate(QKCH):
                for dst, wofs, eng in ((qT, 0, 0), (kT, 8, 1)):
                    pq = ps.tile([128, 512], F32, tag="s", name="pq")
                    for h in range(HLOC):
                        nc.tensor.matmul(
                            pq[32 * h : 32 * h + 2, :cn],
                            w24_sb[:, wofs + 2 * h : wofs + 2 * h + 2],
                            x_sb[:, co : co + cn],
                            start=True, stop=True,
                            tile_position=(0, 32 * h),
                        )
                    if (ci + eng) % 2 == 0:
                        nc.vector.tensor_copy(dst[:, co : co + cn], pq[:, :cn])
                    else:
                        nc.scalar.copy(dst[:, co : co + cn], pq[:, :cn])

            # ---- V': x key-chunk-pairs as weights -> [128, 8] per pair ----
            # (emitted from qt0's first hook so it doesn't delay the first S)
            def emit_vprime():
                psv = ps.tile([128, NPR * 8], F32, tag="s", name="psv")
                for pr, (ko, kn2) in enumerate(PAIRS):
                    nc.tensor.matmul(
                        psv[:kn2, 8 * pr : 8 * pr + 8],
                        x_sb[:, ko : ko + kn2],
                        w24_sb[:, 16:24],
                        start=True, stop=True,
                    )
                psv_v = psv[:].rearrange("p (pr h d) -> p pr h d", h=HLOC, d=2)
                nc.vector.tensor_copy(vp_v[:, :, :, 0:2], psv_v[:, :, :, :])

            # ---- main attention loops ----
            def emit_S(qo, qn, pr):
                # one MM per head: weight kT[2, kn2] covers the whole key
                # pair; 4 heads row-tiled at (32h, 0) run concurrently.
                ko, kn2 = PAIRS[pr]
                tiles = [ps.tile([128, 512], F32, tag="s", name=f"sc{h}")
                         for h in range(HLOC)]
                for h in range(HLOC):
                    nc.tensor.matmul(
                        tiles[h][:kn2, :qn],
                        kT[32 * h : 32 * h + 2, ko : ko + kn2],
                        qT[32 * h : 32 * h + 2, qo : qo + qn],
                        start=True, stop=True,
                        tile_position=(32 * h, 0),
                    )
                return tiles

            def emit_exp(qn, sc, n_scalar):
                es = [epool.tile([128, 512], BF16, tag="e", name=f"es{h}")
                      for h in range(HLOC)]
                for h in range(HLOC):
                    if h < n_scalar:
                        nc.scalar.activation(
                            es[h][:, :qn], sc[h][:, :qn], EXP, scale=SCALE)
                    else:
                        nc.vector.tensor_scalar(
                            es[h][:, :qn].bitcast(I16), sc[h][:, :qn],
                            A16, B16, MUL, ADD,
                        )
                return es

            def emit_U(qn, pr, es, pu):
                ko, kn2 = PAIRS[pr]
                for h in range(HLOC):
                    nc.tensor.matmul(
                        pu[32 * h : 32 * h + 3, 0:qn],
                        vp_v[:kn2, pr, h, :],
                        es[h][:kn2, :qn],
                        start=(pr == 0), stop=(pr == NPR - 1),
                        tile_position=(0, 32 * h),
                    )

            def emit_divide(qo, qn, pu):
                zz = dpool.tile([128, 512], F32, tag="zz", name="zz")
                zz_v = zz[:, :qn].rearrange("(h g) n -> h g n", g=32)
                nc.vector.reciprocal_approx_fast(zz[:, :qn], pu[:, :qn])
                nc.sync.dma_start(out=zz_v[:, 0, :], in_=zz_v[:, 2, :])
                nc.gpsimd.dma_start(out=zz_v[:, 1, :], in_=zz_v[:, 2, :])
                osp = dpool.tile([128, 512], F32, tag="osp", name="osp")
                nc.vector.tensor_mul(osp[:, :qn], pu[:, :qn], zz[:, :qn])
                ov = osp[:, :qn].rearrange("(h g) n -> h g n", g=32)
                ot_v = ot[0 : 2 * HLOC, qo : qo + qn].rearrange("(h d) n -> h d n", d=2)
                nc.sync.dma_start(out=ot_v[:, 0, :], in_=ov[:, 0, :])
                nc.gpsimd.dma_start(out=ot_v[:, 1, :], in_=ov[:, 1, :])
                return zz

            def emit_proj(tstart, nch, copy_eng=0):
                py = ps.tile([108, 512], F32, tag="s", name="py")
                for t8 in range(nch):
                    t = tstart + t8
                    nc.tensor.matmul(
                        py[:108, 64 * t8 : 64 * t8 + 64],
                        ot[:, 108 * t : 108 * t + 108],
                        wp_sb[:],
                        start=True, stop=True,
                    )
                yb = ybs[tstart // 8]
                o8 = (tstart % 8) * 64
                if copy_eng == 0:
                    nc.vector.tensor_copy(yb[:, o8 : o8 + 64 * nch], py[:108, : 64 * nch])
                else:
                    nc.scalar.copy(yb[:, o8 : o8 + 64 * nch], py[:108, : 64 * nch])

            def emit_proj_out(half):
                yb = ybs[half]
                yv = y[864 * half : 864 * (half + 1), :].rearrange(
                    "(t i) c -> i t c", i=108
                )
                nc.sync.dma_start(out=yv, in_=yb[:].rearrange("p (t c) -> p t c", c=64))

            def qt_loop(qo, qn, hooks):
                pu = psu.tile([128, 512], F32, tag="u", name="pu")
                pend = None
                for pr in range(NPR):
                    sc = emit_S(qo, qn, pr)
                    if pend is not None:
                        emit_U(qn, pend[0], pend[1], pu)
                    hk = hooks.get(pr)
                    if hk:
                        hk()
                    n_scalar = 2
                    es = emit_exp(qn, sc, n_scalar)
                    pend = (pr, es)
                emit_U(qn, pend[0], pend[1], pu)
                return pu

            for qi, (qo, qn) in enumerate(QTS):
                hooks = {}
                if qi == 0:
                    hooks = {0: emit_vprime}
                elif qi == 1:
                    hooks = {4: lambda: emit_proj(0, 2, 0),
                             7: lambda: emit_proj(2, 2, 1)}
                elif qi == 2:
                    hooks = {4: lambda: emit_proj(4, 2, 0),
                             7: lambda: emit_proj(6, 2, 1),
                             10: lambda: emit_proj_out(0)}
                elif qi == 3:
                    hooks = {4: lambda: emit_proj(8, 2, 0),
                             7: lambda: emit_proj(10, 2, 1),
                             10: lambda: emit_proj(12, 2, 0)}
                pu = qt_loop(qo, qn, hooks)
                z = emit_divide(qo, qn, pu)
                if qi == 0 and debug:
                    # dump immediately: the dpool buffer is recycled by later qts
                    nc.sync.dma_start(out=d_u[:], in_=z[:])
            emit_proj(14, 2, 1)
            emit_proj_out(1)

            if debug:
                nc.sync.dma_start(out=d_qT[:], in_=qT[:])
                nc.sync.dma_start(out=d_kT[:], in_=kT[:])
                nc.sync.dma_start(out=d_vp[:], in_=vp[:])
                nc.sync.dma_start(out=d_ot[:], in_=ot[:])

    return nc


_NC = None


def _get_nc():
    global _NC
    if _NC is None:
        _NC = build_nc()
        _NC.finalize()
    return _NC


def make_in_maps(x, w_qkv, w_proj, b_proj):
    x2 = np.ascontiguousarray(x.reshape(C, N)).astype(ml_dtypes.bfloat16)
    in_maps = []
    for c in range(NCORES):
        sl = slice(8 * c, 8 * c + 8)
        w24 = np.concatenate(
            [
                w_qkv[sl, :].T,
                w_qkv[64 + 8 * c : 64 + 8 * c + 8, :].T,
                w_qkv[128 + 8 * c : 128 + 8 * c + 8, :].T,
            ],
            axis=1,
        ).astype(ml_dtypes.bfloat16)
        wp = np.concatenate(
            [w_proj[:, sl].T, (b_proj / NCORES)[None, :]], axis=0
        ).astype(np.float32)
        in_maps.append(
            {
                "x2": x2,
                "w24": np.ascontiguousarray(w24),
                "wp": np.ascontiguousarray(wp),
            }
        )
    return in_maps


def run(x, w_qkv, w_proj, b_proj, trace=False, **kw):
    nc = _get_nc()
    in_maps = make_in_maps(x, w_qkv, w_proj, b_proj)
    res = run_bass_kernel_spmd(
        nc, in_maps, core_ids=list(range(NCORES)), trace=trace, **kw
    )
    y = np.zeros((N, C), np.float32)
    for r in res.results:
        y += r["y"]
    return y.reshape(1, 12, 12, 12, C), res


def kernel(x, w_qkv, w_proj, b_proj):
    out, _ = run(
        np.asarray(x), np.asarray(w_qkv), np.asarray(w_proj), np.asarray(b_proj)
    )
    return out


# revision 7
# speedup vs baseline: 2.1680x; 1.0606x over previous
"""Trainium2 Bass kernel for nn_Attention (B=1, C=64, 12x12x12 spatial, 32 heads, head_dim=2).

Sharding: 32 heads over 8 cores (4 heads/core), tensor-parallel: per-core
partial output summed on host with bias/8 folded per core.

v5 vs v4 (127us): 2+2 exp split, V-prime off the prologue critical path,
interleaved q/k staging, proj 12-13 overlapped into qt3.
- S-matmuls run 8-way concurrent: (32h, 64*parity) for 4 heads x 2 x 64-key
  chunks = all 16 PE sub-arrays; each pair-iter's S wall is ~qn columns.
- U-matmuls contract 128 keys per pass (halves U column-streams), 4-way
  col-tiled at (0, 32h).
- Two heads share one [128, 1024] f32 score tile (h at col offset qn*(h%2))
  so each exp instruction runs at FD=2*qn (amortizes engine overhead).
- Hybrid exp: ScalarE ACT exp for tile AB (+ every 4th CD), DVE Schraudolph
  (tensor_scalar f32->i16; bits are the bf16 exp approx) for the rest.
- Query tiles of 512 (PSUM bank limit): tiles AB/CD are 2 banks each,
  pool of 3 + pu 1 bank = 7 of 8 banks.
"""

import numpy as np
import ml_dtypes

import concourse.bass as bass
import concourse.bacc as bacc
import concourse.mybir as mybir
from concourse import tile
from concourse.bass_utils import run_bass_kernel_spmd

C = 64
N = 1728
NCORES = 8
HLOC = 4
SCALE = float(2.0 ** -0.5)

QTS = [(0, 512), (512, 512), (1024, 512), (1536, 192)]
PAIRS = [(128 * p, 128) for p in range(13)] + [(1664, 64)]
NPR = len(PAIRS)
QKCH = [(0, 512), (512, 512), (1024, 512), (1536, 192)]

LOG2E = 1.4426950408889634
A16 = SCALE * 128.0 * LOG2E
B16 = 127.0 * 128.0

F32 = mybir.dt.float32
BF16 = mybir.dt.bfloat16
I16 = mybir.dt.int16
EXP = mybir.ActivationFunctionType.Exp
MUL = mybir.AluOpType.mult
ADD = mybir.AluOpType.add


def build_nc(debug=False):
    nc = bacc.Bacc(None)

    x2 = nc.declare_dram_parameter("x2", [C, N], BF16, isOutput=False)
    w24 = nc.declare_dram_parameter("w24", [C, 3 * 2 * HLOC], BF16, isOutput=False)
    wp = nc.declare_dram_parameter("wp", [2 * HLOC + 1, C], F32, isOutput=False)
    y = nc.declare_dram_parameter("y", [N, C], F32, isOutput=True)
    if debug:
        d_qT = nc.declare_dram_parameter("d_qT", [128, N], BF16, isOutput=True)
        d_kT = nc.declare_dram_parameter("d_kT", [128, N], BF16, isOutput=True)
        d_vp = nc.declare_dram_parameter("d_vp", [128, NPR * 12], BF16, isOutput=True)
        d_ot = nc.declare_dram_parameter("d_ot", [9, N], F32, isOutput=True)
        d_u = nc.declare_dram_parameter("d_u", [128, 512], F32, isOutput=True)

    with tile.TileContext(nc) as tc:
        with (
            tc.tile_pool(name="const", bufs=1) as cpool,
            tc.tile_pool(name="epool", bufs=10) as epool,
            tc.tile_pool(name="dpool", bufs=2) as dpool,
            tc.tile_pool(name="ps", bufs=7, space=bass.MemorySpace.PSUM) as ps,
            tc.tile_pool(name="psu", bufs=1, space=bass.MemorySpace.PSUM) as psu,
        ):
            x_sb = cpool.tile([C, N], BF16, name="x_sb")
            w24_sb = cpool.tile([C, 24], BF16, name="w24_sb")
            wp_sb = cpool.tile([2 * HLOC + 1, C], F32, name="wp_sb")
            qT = cpool.tile([128, N], BF16, name="qT")
            kT = cpool.tile([128, N], BF16, name="kT")
            vp = cpool.tile([128, NPR * HLOC * 3], BF16, name="vp")
            ot = cpool.tile([2 * HLOC + 1, N], F32, name="ot")
            yb0 = cpool.tile([108, 512], F32, name="yb0")
            yb1 = cpool.tile([108, 512], F32, name="yb1")
            ybs = [yb0, yb1]

            nc.sync.dma_start(out=x_sb[:], in_=x2[:])
            nc.sync.dma_start(out=w24_sb[:], in_=w24[:])
            nc.sync.dma_start(out=wp_sb[:], in_=wp[:])

            vp_v = vp[:].rearrange("p (pr h c) -> p pr h c", h=HLOC, c=3)
            # ot rows 0..7 are overwritten by the divide unpack; row 8 stays 1.0
            nc.gpsimd.memset(ot[:, :], 1.0)
            nc.gpsimd.memset(vp_v[:, :, :, 2:3], 1.0)

            # ---- qkv projections, col-tiled 4 heads at once ----
            for ci, (co, cn) in enumerate(QKCH):
                for dst, wofs, eng in ((qT, 0, 0), (kT, 8, 1)):
                    pq = ps.tile([128, 512], F32, tag="s", name="pq")
                    for h in range(HLOC):
                        nc.tensor.matmul(
                            pq[32 * h : 32 * h + 2, :cn],
                            w24_sb[:, wofs + 2 * h : wofs + 2 * h + 2],
                            x_sb[:, co : co + cn],
                            start=True, stop=True,
                            tile_position=(0, 32 * h),
                        )
                    if (ci + eng) % 2 == 0:
                        nc.vector.tensor_copy(dst[:, co : co + cn], pq[:, :cn])
                    else:
                        nc.scalar.copy(dst[:, co : co + cn], pq[:, :cn])

            # ---- V': x key-chunk-pairs as weights -> [128, 8] per pair ----
            # (emitted from qt0's first hook so it doesn't delay the first S)
            def emit_vprime():
                psv = ps.tile([128, NPR * 8], F32, tag="s", name="psv")
                for pr, (ko, kn2) in enumerate(PAIRS):
                    nc.tensor.matmul(
                        psv[:kn2, 8 * pr : 8 * pr + 8],
                        x_sb[:, ko : ko + kn2],
                        w24_sb[:, 16:24],
                        start=True, stop=True,
                    )
                psv_v = psv[:].rearrange("p (pr h d) -> p pr h d", h=HLOC, d=2)
                nc.vector.tensor_copy(vp_v[:, :, :, 0:2], psv_v[:, :, :, :])

            # ---- main attention loops ----
            def emit_S(qo, qn, pr):
                # one MM per head: weight kT[2, kn2] covers the whole key
                # pair; 4 heads row-tiled at (32h, 0) run concurrently.
                ko, kn2 = PAIRS[pr]
                tiles = [ps.tile([128, 512], F32, tag="s", name=f"sc{h}")
                         for h in range(HLOC)]
                for h in range(HLOC):
                    nc.tensor.matmul(
                        tiles[h][:kn2, :qn],
                        kT[32 * h : 32 * h + 2, ko : ko + kn2],
                        qT[32 * h : 32 * h + 2, qo : qo + qn],
                        start=True, stop=True,
                        tile_position=(32 * h, 0),
                    )
                return tiles

            def emit_exp(qn, sc, n_scalar):
                es = [epool.tile([128, 512], BF16, tag="e", name=f"es{h}")
                      for h in range(HLOC)]
                for h in range(HLOC):
                    if h < n_scalar:
                        nc.scalar.activation(
                            es[h][:, :qn], sc[h][:, :qn], EXP, scale=SCALE)
                    else:
                        nc.vector.tensor_scalar(
                            es[h][:, :qn].bitcast(I16), sc[h][:, :qn],
                            A16, B16, MUL, ADD,
                        )
                return es

            def emit_U(qn, pr, es, pu):
                ko, kn2 = PAIRS[pr]
                for h in range(HLOC):
                    nc.tensor.matmul(
                        pu[32 * h : 32 * h + 3, 0:qn],
                        vp_v[:kn2, pr, h, :],
                        es[h][:kn2, :qn],
                        start=(pr == 0), stop=(pr == NPR - 1),
                        tile_position=(0, 32 * h),
                    )

            def emit_divide(qo, qn, pu):
                zz = dpool.tile([128, 512], F32, tag="zz", name="zz")
                zz_v = zz[:, :qn].rearrange("(h g) n -> h g n", g=32)
                nc.vector.reciprocal_approx_fast(zz[:, :qn], pu[:, :qn])
                nc.sync.dma_start(out=zz_v[:, 0, :], in_=zz_v[:, 2, :])
                nc.gpsimd.dma_start(out=zz_v[:, 1, :], in_=zz_v[:, 2, :])
                osp = dpool.tile([128, 512], F32, tag="osp", name="osp")
                nc.vector.tensor_mul(osp[:, :qn], pu[:, :qn], zz[:, :qn])
                ov = osp[:, :qn].rearrange("(h g) n -> h g n", g=32)
                ot_v = ot[0 : 2 * HLOC, qo : qo + qn].rearrange("(h d) n -> h d n", d=2)
                nc.sync.dma_start(out=ot_v[:, 0, :], in_=ov[:, 0, :])
                nc.gpsimd.dma_start(out=ot_v[:, 1, :], in_=ov[:, 1, :])
                return zz

            def emit_proj(tstart, nch, copy_eng=0):
                py = ps.tile([108, 512], F32, tag="s", name="py")
                for t8 in range(nch):
                    t = tstart + t8
                    nc.tensor.matmul(
                        py[:108, 64 * t8 : 64 * t8 + 64],
                        ot[:, 108 * t : 108 * t + 108],
                        wp_sb[:],
                        start=True, stop=True,
                    )
                yb = ybs[tstart // 8]
                o8 = (tstart % 8) * 64
                if copy_eng == 0:
                    nc.vector.tensor_copy(yb[:, o8 : o8 + 64 * nch], py[:108, : 64 * nch])
                else:
                    nc.scalar.copy(yb[:, o8 : o8 + 64 * nch], py[:108, : 64 * nch])

            def emit_proj_out(half):
                yb = ybs[half]
                yv = y[864 * half : 864 * (half + 1), :].rearrange(
                    "(t i) c -> i t c", i=108
                )
                nc.sync.dma_start(out=yv, in_=yb[:].rearrange("p (t c) -> p t c", c=64))

            def qt_loop(qo, qn, hooks):
                pu = psu.tile([128, 512], F32, tag="u", name="pu")
                pend = None
                for pr in range(NPR):
                    sc = emit_S(qo, qn, pr)
                    if pend is not None:
                        emit_U(qn, pend[0], pend[1], pu)
                    hk = hooks.get(pr)
                    if hk:
                        hk()
                    n_scalar = 2
                    es = emit_exp(qn, sc, n_scalar)
                    pend = (pr, es)
                emit_U(qn, pend[0], pend[1], pu)
                return pu

            for qi, (qo, qn) in enumerate(QTS):
                hooks = {}
                if qi == 0:
                    hooks = {0: emit_vprime}
                elif qi == 1:
                    hooks = {4: lambda: emit_proj(0, 2, 0),
                             7: lambda: emit_proj(2, 2, 1)}
                elif qi == 2:
                    hooks = {4: lambda: emit_proj(4, 2, 0),
                             7: lambda: emit_proj(6, 2, 1),
                             10: lambda: emit_proj_out(0)}
                elif qi == 3:
                    hooks = {4: lambda: emit_proj(8, 2, 0),
                             7: lambda: emit_proj(10, 2, 1),
                             10: lambda: emit_proj(12, 2, 0)}
                pu = qt_loop(qo, qn, hooks)
                z = emit_divide(qo, qn, pu)
                if qi == 0 and debug:
                    # dump immediately: the dpool buffer is recycled by later qts
                    nc.sync.dma_start(out=d_u[:], in_=z[:])
            emit_proj(14, 2, 1)
            emit_proj_out(1)

            if debug:
                nc.sync.dma_start(out=d_qT[:], in_=qT[:])
                nc.sync.dma_start(out=d_kT[:], in_=kT[:])
                nc.sync.dma_start(out=d_vp[:], in_=vp[:])
                nc.sync.dma_start(out=d_ot[:], in_=ot[:])

    return nc


_NC = None


def _get_nc():
    global _NC
    if _NC is None:
        _NC = build_nc()
        _NC.finalize()
    return _NC


def make_in_maps(x, w_qkv, w_proj, b_proj):
    x2 = np.ascontiguousarray(x.reshape(C, N)).astype(ml_dtypes.bfloat16)
    in_maps = []
    for c in range(NCORES):
        sl = slice(8 * c, 8 * c + 8)
        w24 = np.concatenate(
            [
                w_qkv[sl, :].T,
                w_qkv[64 + 8 * c : 64 + 8 * c + 8, :].T,
                w_qkv[128 + 8 * c : 128 + 8 * c + 8, :].T,
            ],
            axis=1,
        ).astype(ml_dtypes.bfloat16)
        wp = np.concatenate(
            [w_proj[:, sl].T, (b_proj / NCORES)[None, :]], axis=0
        ).astype(np.float32)
        in_maps.append(
            {
                "x2": x2,
                "w24": np.ascontiguousarray(w24),
                "wp": np.ascontiguousarray(wp),
            }
        )
    return in_maps


def run(x, w_qkv, w_proj, b_proj, trace=False, **kw):
    nc = _get_nc()
    in_maps = make_in_maps(x, w_qkv, w_proj, b_proj)
    res = run_bass_kernel_spmd(
        nc, in_maps, core_ids=list(range(NCORES)), trace=trace, **kw
    )
    y = np.zeros((N, C), np.float32)
    for r in res.results:
        y += r["y"]
    return y.reshape(1, 12, 12, 12, C), res


def kernel(x, w_qkv, w_proj, b_proj):
    out, _ = run(
        np.asarray(x), np.asarray(w_qkv), np.asarray(w_proj), np.asarray(b_proj)
    )
    return out


# revision 8
# speedup vs baseline: 2.1947x; 1.0123x over previous
"""Trainium2 Bass kernel for nn_Attention (B=1, C=64, 12x12x12 spatial, 32 heads, head_dim=2).

Sharding: 32 heads over 8 cores (4 heads/core), tensor-parallel: per-core
partial output summed on host with bias/8 folded per core.

v5 vs v4 (127us): 2+2 exp split, V-prime off the prologue critical path,
interleaved q/k staging, proj 12-13 overlapped into qt3.
- S-matmuls run 8-way concurrent: (32h, 64*parity) for 4 heads x 2 x 64-key
  chunks = all 16 PE sub-arrays; each pair-iter's S wall is ~qn columns.
- U-matmuls contract 128 keys per pass (halves U column-streams), 4-way
  col-tiled at (0, 32h).
- Two heads share one [128, 1024] f32 score tile (h at col offset qn*(h%2))
  so each exp instruction runs at FD=2*qn (amortizes engine overhead).
- Hybrid exp: ScalarE ACT exp for tile AB (+ every 4th CD), DVE Schraudolph
  (tensor_scalar f32->i16; bits are the bf16 exp approx) for the rest.
- Query tiles of 512 (PSUM bank limit): tiles AB/CD are 2 banks each,
  pool of 3 + pu 1 bank = 7 of 8 banks.
"""

import numpy as np
import ml_dtypes

import concourse.bass as bass
import concourse.bacc as bacc
import concourse.mybir as mybir
from concourse import tile
from concourse.bass_utils import run_bass_kernel_spmd

C = 64
N = 1728
NCORES = 8
HLOC = 4
SCALE = float(2.0 ** -0.5)

QTS = [(0, 512), (512, 512), (1024, 512), (1536, 192)]
PAIRS = [(128 * p, 128) for p in range(13)] + [(1664, 64)]
NPR = len(PAIRS)
QKCH = [(0, 512), (512, 512), (1024, 512), (1536, 192)]

LOG2E = 1.4426950408889634
A16 = SCALE * 128.0 * LOG2E
B16 = 127.0 * 128.0

F32 = mybir.dt.float32
BF16 = mybir.dt.bfloat16
I16 = mybir.dt.int16
EXP = mybir.ActivationFunctionType.Exp
MUL = mybir.AluOpType.mult
ADD = mybir.AluOpType.add


def build_nc(debug=False):
    nc = bacc.Bacc(None)

    x2 = nc.declare_dram_parameter("x2", [C, N], BF16, isOutput=False)
    w24 = nc.declare_dram_parameter("w24", [C, 3 * 2 * HLOC], BF16, isOutput=False)
    wp = nc.declare_dram_parameter("wp", [2 * HLOC + 1, C], F32, isOutput=False)
    y = nc.declare_dram_parameter("y", [N, C], F32, isOutput=True)
    if debug:
        d_qT = nc.declare_dram_parameter("d_qT", [128, N], BF16, isOutput=True)
        d_kT = nc.declare_dram_parameter("d_kT", [128, N], BF16, isOutput=True)
        d_vp = nc.declare_dram_parameter("d_vp", [128, NPR * 12], BF16, isOutput=True)
        d_ot = nc.declare_dram_parameter("d_ot", [9, N], F32, isOutput=True)
        d_u = nc.declare_dram_parameter("d_u", [128, 512], F32, isOutput=True)

    with tile.TileContext(nc) as tc:
        with (
            tc.tile_pool(name="const", bufs=1) as cpool,
            tc.tile_pool(name="epool", bufs=12) as epool,
            tc.tile_pool(name="dpool", bufs=2) as dpool,
            tc.tile_pool(name="ps", bufs=7, space=bass.MemorySpace.PSUM) as ps,
            tc.tile_pool(name="psu", bufs=1, space=bass.MemorySpace.PSUM) as psu,
        ):
            x_sb = cpool.tile([C, N], BF16, name="x_sb")
            w24_sb = cpool.tile([C, 24], BF16, name="w24_sb")
            wp_sb = cpool.tile([2 * HLOC + 1, C], F32, name="wp_sb")
            qT = cpool.tile([128, N], BF16, name="qT")
            kT = cpool.tile([128, N], BF16, name="kT")
            vp = cpool.tile([128, NPR * HLOC * 3], BF16, name="vp")
            ot = cpool.tile([2 * HLOC + 1, N], F32, name="ot")
            yb0 = cpool.tile([108, 512], F32, name="yb0")
            yb1 = cpool.tile([108, 512], F32, name="yb1")
            ybs = [yb0, yb1]

            nc.sync.dma_start(out=x_sb[:], in_=x2[:])
            nc.sync.dma_start(out=w24_sb[:], in_=w24[:])
            nc.sync.dma_start(out=wp_sb[:], in_=wp[:])

            vp_v = vp[:].rearrange("p (pr h c) -> p pr h c", h=HLOC, c=3)
            # ot rows 0..7 are overwritten by the divide unpack; row 8 stays 1.0
            nc.gpsimd.memset(ot[:, :], 1.0)
            nc.gpsimd.memset(vp_v[:, :, :, 2:3], 1.0)

            # ---- qkv projections, col-tiled 4 heads at once ----
            for ci, (co, cn) in enumerate(QKCH):
                for dst, wofs, eng in ((qT, 0, 0), (kT, 8, 1)):
                    pq = ps.tile([128, 512], F32, tag="s", name="pq")
                    for h in range(HLOC):
                        nc.tensor.matmul(
                            pq[32 * h : 32 * h + 2, :cn],
                            w24_sb[:, wofs + 2 * h : wofs + 2 * h + 2],
                            x_sb[:, co : co + cn],
                            start=True, stop=True,
                            tile_position=(0, 32 * h),
                        )
                    if (ci + eng) % 2 == 0:
                        nc.vector.tensor_copy(dst[:, co : co + cn], pq[:, :cn])
                    else:
                        nc.scalar.copy(dst[:, co : co + cn], pq[:, :cn])

            # ---- V': x key-chunk-pairs as weights -> [128, 8] per pair ----
            # (emitted from qt0's first hook so it doesn't delay the first S)
            def emit_vprime():
                psv = ps.tile([128, NPR * 8], F32, tag="s", name="psv")
                for pr, (ko, kn2) in enumerate(PAIRS):
                    nc.tensor.matmul(
                        psv[:kn2, 8 * pr : 8 * pr + 8],
                        x_sb[:, ko : ko + kn2],
                        w24_sb[:, 16:24],
                        start=True, stop=True,
                    )
                psv_v = psv[:].rearrange("p (pr h d) -> p pr h d", h=HLOC, d=2)
                nc.vector.tensor_copy(vp_v[:, :, :, 0:2], psv_v[:, :, :, :])

            # ---- main attention loops ----
            def emit_S(qo, qn, pr):
                # one MM per head: weight kT[2, kn2] covers the whole key
                # pair; 4 heads row-tiled at (32h, 0) run concurrently.
                ko, kn2 = PAIRS[pr]
                tiles = [ps.tile([128, 512], F32, tag="s", name=f"sc{h}")
                         for h in range(HLOC)]
                for h in range(HLOC):
                    nc.tensor.matmul(
                        tiles[h][:kn2, :qn],
                        kT[32 * h : 32 * h + 2, ko : ko + kn2],
                        qT[32 * h : 32 * h + 2, qo : qo + qn],
                        start=True, stop=True,
                        tile_position=(32 * h, 0),
                    )
                return tiles

            def emit_exp(qn, sc, n_scalar):
                es = [epool.tile([128, 512], BF16, tag="e", name=f"es{h}")
                      for h in range(HLOC)]
                for h in range(HLOC):
                    if h < n_scalar:
                        nc.scalar.activation(
                            es[h][:, :qn], sc[h][:, :qn], EXP, scale=SCALE)
                    else:
                        nc.vector.tensor_scalar(
                            es[h][:, :qn].bitcast(I16), sc[h][:, :qn],
                            A16, B16, MUL, ADD,
                        )
                return es

            def emit_U(qn, pr, es, pu):
                ko, kn2 = PAIRS[pr]
                for h in range(HLOC):
                    nc.tensor.matmul(
                        pu[32 * h : 32 * h + 3, 0:qn],
                        vp_v[:kn2, pr, h, :],
                        es[h][:kn2, :qn],
                        start=(pr == 0), stop=(pr == NPR - 1),
                        tile_position=(0, 32 * h),
                    )

            def emit_divide(qo, qn, pu):
                zz = dpool.tile([128, 512], F32, tag="zz", name="zz")
                zz_v = zz[:, :qn].rearrange("(h g) n -> h g n", g=32)
                nc.vector.reciprocal_approx_fast(zz[:, :qn], pu[:, :qn])
                nc.sync.dma_start(out=zz_v[:, 0, :], in_=zz_v[:, 2, :])
                nc.gpsimd.dma_start(out=zz_v[:, 1, :], in_=zz_v[:, 2, :])
                osp = dpool.tile([128, 512], F32, tag="osp", name="osp")
                nc.vector.tensor_mul(osp[:, :qn], pu[:, :qn], zz[:, :qn])
                ov = osp[:, :qn].rearrange("(h g) n -> h g n", g=32)
                ot_v = ot[0 : 2 * HLOC, qo : qo + qn].rearrange("(h d) n -> h d n", d=2)
                nc.sync.dma_start(out=ot_v[:, 0, :], in_=ov[:, 0, :])
                nc.gpsimd.dma_start(out=ot_v[:, 1, :], in_=ov[:, 1, :])
                return zz

            def emit_proj(tstart, nch, copy_eng=0):
                py = ps.tile([108, 512], F32, tag="s", name="py")
                for t8 in range(nch):
                    t = tstart + t8
                    nc.tensor.matmul(
                        py[:108, 64 * t8 : 64 * t8 + 64],
                        ot[:, 108 * t : 108 * t + 108],
                        wp_sb[:],
                        start=True, stop=True,
                    )
                yb = ybs[tstart // 8]
                o8 = (tstart % 8) * 64
                if copy_eng == 0:
                    nc.vector.tensor_copy(yb[:, o8 : o8 + 64 * nch], py[:108, : 64 * nch])
                else:
                    nc.scalar.copy(yb[:, o8 : o8 + 64 * nch], py[:108, : 64 * nch])

            def emit_proj_out(half):
                yb = ybs[half]
                yv = y[864 * half : 864 * (half + 1), :].rearrange(
                    "(t i) c -> i t c", i=108
                )
                nc.sync.dma_start(out=yv, in_=yb[:].rearrange("p (t c) -> p t c", c=64))

            def qt_loop(qo, qn, hooks):
                pu = psu.tile([128, 512], F32, tag="u", name="pu")
                pend = []
                for pr in range(NPR):
                    sc = emit_S(qo, qn, pr)
                    # lag U by TWO iters so its es inputs are never fresh
                    # (U-group leader otherwise stalls on the exp engines)
                    if len(pend) == 2:
                        p0 = pend.pop(0)
                        emit_U(qn, p0[0], p0[1], pu)
                    hk = hooks.get(pr)
                    if hk:
                        hk()
                    es = emit_exp(qn, sc, 2)
                    pend.append((pr, es))
                for p0 in pend:
                    emit_U(qn, p0[0], p0[1], pu)
                return pu

            for qi, (qo, qn) in enumerate(QTS):
                hooks = {}
                if qi == 0:
                    hooks = {0: emit_vprime}
                elif qi == 1:
                    hooks = {4: lambda: emit_proj(0, 2, 0),
                             7: lambda: emit_proj(2, 2, 1)}
                elif qi == 2:
                    hooks = {4: lambda: emit_proj(4, 2, 0),
                             7: lambda: emit_proj(6, 2, 1),
                             10: lambda: emit_proj_out(0)}
                elif qi == 3:
                    hooks = {4: lambda: emit_proj(8, 2, 0),
                             7: lambda: emit_proj(10, 2, 1),
                             10: lambda: emit_proj(12, 2, 0)}
                pu = qt_loop(qo, qn, hooks)
                z = emit_divide(qo, qn, pu)
                if qi == 0 and debug:
                    # dump immediately: the dpool buffer is recycled by later qts
                    nc.sync.dma_start(out=d_u[:], in_=z[:])
            emit_proj(14, 2, 1)
            emit_proj_out(1)

            if debug:
                nc.sync.dma_start(out=d_qT[:], in_=qT[:])
                nc.sync.dma_start(out=d_kT[:], in_=kT[:])
                nc.sync.dma_start(out=d_vp[:], in_=vp[:])
                nc.sync.dma_start(out=d_ot[:], in_=ot[:])

    return nc


_NC = None


def _get_nc():
    global _NC
    if _NC is None:
        _NC = build_nc()
        _NC.finalize()
    return _NC


def make_in_maps(x, w_qkv, w_proj, b_proj):
    x2 = np.ascontiguousarray(x.reshape(C, N)).astype(ml_dtypes.bfloat16)
    in_maps = []
    for c in range(NCORES):
        sl = slice(8 * c, 8 * c + 8)
        w24 = np.concatenate(
            [
                w_qkv[sl, :].T,
                w_qkv[64 + 8 * c : 64 + 8 * c + 8, :].T,
                w_qkv[128 + 8 * c : 128 + 8 * c + 8, :].T,
            ],
            axis=1,
        ).astype(ml_dtypes.bfloat16)
        wp = np.concatenate(
            [w_proj[:, sl].T, (b_proj / NCORES)[None, :]], axis=0
        ).astype(np.float32)
        in_maps.append(
            {
                "x2": x2,
                "w24": np.ascontiguousarray(w24),
                "wp": np.ascontiguousarray(wp),
            }
        )
    return in_maps


def run(x, w_qkv, w_proj, b_proj, trace=False, **kw):
    nc = _get_nc()
    in_maps = make_in_maps(x, w_qkv, w_proj, b_proj)
    res = run_bass_kernel_spmd(
        nc, in_maps, core_ids=list(range(NCORES)), trace=trace, **kw
    )
    y = np.zeros((N, C), np.float32)
    for r in res.results:
        y += r["y"]
    return y.reshape(1, 12, 12, 12, C), res


def kernel(x, w_qkv, w_proj, b_proj):
    out, _ = run(
        np.asarray(x), np.asarray(w_qkv), np.asarray(w_proj), np.asarray(b_proj)
    )
    return out


# revision 9
# speedup vs baseline: 2.1990x; 1.0020x over previous
"""Trainium2 Bass kernel for nn_Attention (B=1, C=64, 12x12x12 spatial, 32 heads, head_dim=2).

Sharding: 32 heads over 8 cores (4 heads/core), tensor-parallel: per-core
partial output summed on host with bias/8 folded per core.

v5 vs v4 (127us): 2+2 exp split, V-prime off the prologue critical path,
interleaved q/k staging, proj 12-13 overlapped into qt3.
- S-matmuls run 8-way concurrent: (32h, 64*parity) for 4 heads x 2 x 64-key
  chunks = all 16 PE sub-arrays; each pair-iter's S wall is ~qn columns.
- U-matmuls contract 128 keys per pass (halves U column-streams), 4-way
  col-tiled at (0, 32h).
- Two heads share one [128, 1024] f32 score tile (h at col offset qn*(h%2))
  so each exp instruction runs at FD=2*qn (amortizes engine overhead).
- Hybrid exp: ScalarE ACT exp for tile AB (+ every 4th CD), DVE Schraudolph
  (tensor_scalar f32->i16; bits are the bf16 exp approx) for the rest.
- Query tiles of 512 (PSUM bank limit): tiles AB/CD are 2 banks each,
  pool of 3 + pu 1 bank = 7 of 8 banks.
"""

import numpy as np
import ml_dtypes

import concourse.bass as bass
import concourse.bacc as bacc
import concourse.mybir as mybir
from concourse import tile
from concourse.bass_utils import run_bass_kernel_spmd

C = 64
N = 1728
NCORES = 8
HLOC = 4
SCALE = float(2.0 ** -0.5)

QTS = [(0, 512), (512, 512), (1024, 512), (1536, 192)]
PAIRS = [(128 * p, 128) for p in range(13)] + [(1664, 64)]
NPR = len(PAIRS)
QKCH = [(0, 512), (512, 512), (1024, 512), (1536, 192)]

LOG2E = 1.4426950408889634
A16 = SCALE * 128.0 * LOG2E
B16 = 127.0 * 128.0

F32 = mybir.dt.float32
BF16 = mybir.dt.bfloat16
I16 = mybir.dt.int16
EXP = mybir.ActivationFunctionType.Exp
MUL = mybir.AluOpType.mult
ADD = mybir.AluOpType.add


def build_nc(debug=False):
    nc = bacc.Bacc(None)

    x2 = nc.declare_dram_parameter("x2", [C, N], BF16, isOutput=False)
    w24 = nc.declare_dram_parameter("w24", [C, 3 * 2 * HLOC], BF16, isOutput=False)
    wp = nc.declare_dram_parameter("wp", [2 * HLOC + 1, C], F32, isOutput=False)
    y = nc.declare_dram_parameter("y", [N, C], F32, isOutput=True)
    if debug:
        d_qT = nc.declare_dram_parameter("d_qT", [128, N], BF16, isOutput=True)
        d_kT = nc.declare_dram_parameter("d_kT", [128, N], BF16, isOutput=True)
        d_vp = nc.declare_dram_parameter("d_vp", [128, NPR * 12], BF16, isOutput=True)
        d_ot = nc.declare_dram_parameter("d_ot", [9, N], F32, isOutput=True)
        d_u = nc.declare_dram_parameter("d_u", [128, 512], F32, isOutput=True)

    with tile.TileContext(nc) as tc:
        with (
            tc.tile_pool(name="const", bufs=1) as cpool,
            tc.tile_pool(name="epool", bufs=12) as epool,
            tc.tile_pool(name="dpool", bufs=2) as dpool,
            tc.tile_pool(name="ps", bufs=7, space=bass.MemorySpace.PSUM) as ps,
            tc.tile_pool(name="psu", bufs=1, space=bass.MemorySpace.PSUM) as psu,
        ):
            x_sb = cpool.tile([C, N], BF16, name="x_sb")
            w24_sb = cpool.tile([C, 24], BF16, name="w24_sb")
            wp_sb = cpool.tile([2 * HLOC + 1, C], F32, name="wp_sb")
            qT = cpool.tile([128, N], BF16, name="qT")
            kT = cpool.tile([128, N], BF16, name="kT")
            vp = cpool.tile([128, NPR * HLOC * 3], BF16, name="vp")
            ot = cpool.tile([2 * HLOC + 1, N], F32, name="ot")
            yb0 = cpool.tile([108, 512], F32, name="yb0")
            yb1 = cpool.tile([108, 512], F32, name="yb1")
            ybs = [yb0, yb1]

            nc.sync.dma_start(out=x_sb[:], in_=x2[:])
            nc.sync.dma_start(out=w24_sb[:], in_=w24[:])
            nc.sync.dma_start(out=wp_sb[:], in_=wp[:])

            vp_v = vp[:].rearrange("p (pr h c) -> p pr h c", h=HLOC, c=3)
            # ot rows 0..7 are overwritten by the divide unpack; row 8 stays 1.0
            nc.gpsimd.memset(ot[:, :], 1.0)
            nc.gpsimd.memset(vp_v[:, :, :, 2:3], 1.0)

            # ---- qkv projections, col-tiled 4 heads at once ----
            for ci, (co, cn) in enumerate(QKCH):
                for dst, wofs, eng in ((qT, 0, 0), (kT, 8, 1)):
                    pq = ps.tile([128, 512], F32, tag="s", name="pq")
                    for h in range(HLOC):
                        nc.tensor.matmul(
                            pq[32 * h : 32 * h + 2, :cn],
                            w24_sb[:, wofs + 2 * h : wofs + 2 * h + 2],
                            x_sb[:, co : co + cn],
                            start=True, stop=True,
                            tile_position=(0, 32 * h),
                        )
                    if (ci + eng) % 2 == 0:
                        nc.vector.tensor_copy(dst[:, co : co + cn], pq[:, :cn])
                    else:
                        nc.scalar.copy(dst[:, co : co + cn], pq[:, :cn])

            # ---- V': x key-chunk-pairs as weights -> [128, 8] per pair ----
            # (emitted from qt0's first hook so it doesn't delay the first S)
            def emit_vprime():
                psv = ps.tile([128, NPR * 8], F32, tag="s", name="psv")
                for pr, (ko, kn2) in enumerate(PAIRS):
                    nc.tensor.matmul(
                        psv[:kn2, 8 * pr : 8 * pr + 8],
                        x_sb[:, ko : ko + kn2],
                        w24_sb[:, 16:24],
                        start=True, stop=True,
                    )
                psv_v = psv[:].rearrange("p (pr h d) -> p pr h d", h=HLOC, d=2)
                nc.vector.tensor_copy(vp_v[:, :, :, 0:2], psv_v[:, :, :, :])

            # ---- main attention loops ----
            def emit_S(qo, qn, pr):
                # one MM per head: weight kT[2, kn2] covers the whole key
                # pair; 4 heads row-tiled at (32h, 0) run concurrently.
                ko, kn2 = PAIRS[pr]
                tiles = [ps.tile([128, 512], F32, tag="s", name=f"sc{h}")
                         for h in range(HLOC)]
                for h in range(HLOC):
                    nc.tensor.matmul(
                        tiles[h][:kn2, :qn],
                        kT[32 * h : 32 * h + 2, ko : ko + kn2],
                        qT[32 * h : 32 * h + 2, qo : qo + qn],
                        start=True, stop=True,
                        tile_position=(32 * h, 0),
                    )
                return tiles

            def emit_exp(qn, sc, n_scalar):
                es = [epool.tile([128, 512], BF16, tag="e", name=f"es{h}")
                      for h in range(HLOC)]
                for h in range(HLOC):
                    if h < n_scalar:
                        nc.scalar.activation(
                            es[h][:, :qn], sc[h][:, :qn], EXP, scale=SCALE)
                    else:
                        nc.vector.tensor_scalar(
                            es[h][:, :qn].bitcast(I16), sc[h][:, :qn],
                            A16, B16, MUL, ADD,
                        )
                return es

            def emit_U(qn, pr, es, pu):
                ko, kn2 = PAIRS[pr]
                for h in range(HLOC):
                    nc.tensor.matmul(
                        pu[32 * h : 32 * h + 3, 0:qn],
                        vp_v[:kn2, pr, h, :],
                        es[h][:kn2, :qn],
                        start=(pr == 0), stop=(pr == NPR - 1),
                        tile_position=(0, 32 * h),
                    )

            def emit_divide(qo, qn, pu):
                zz = dpool.tile([128, 512], F32, tag="zz", name="zz")
                zz_v = zz[:, :qn].rearrange("(h g) n -> h g n", g=32)
                nc.vector.reciprocal_approx_fast(zz[:, :qn], pu[:, :qn])
                nc.sync.dma_start(out=zz_v[:, 0, :], in_=zz_v[:, 2, :])
                nc.gpsimd.dma_start(out=zz_v[:, 1, :], in_=zz_v[:, 2, :])
                osp = dpool.tile([128, 512], F32, tag="osp", name="osp")
                nc.vector.tensor_mul(osp[:, :qn], pu[:, :qn], zz[:, :qn])
                ov = osp[:, :qn].rearrange("(h g) n -> h g n", g=32)
                ot_v = ot[0 : 2 * HLOC, qo : qo + qn].rearrange("(h d) n -> h d n", d=2)
                nc.sync.dma_start(out=ot_v[:, 0, :], in_=ov[:, 0, :])
                nc.gpsimd.dma_start(out=ot_v[:, 1, :], in_=ov[:, 1, :])
                return zz

            def emit_proj(tstart, nch, copy_eng=0):
                py = ps.tile([108, 512], F32, tag="s", name="py")
                for t8 in range(nch):
                    t = tstart + t8
                    nc.tensor.matmul(
                        py[:108, 64 * t8 : 64 * t8 + 64],
                        ot[:, 108 * t : 108 * t + 108],
                        wp_sb[:],
                        start=True, stop=True,
                    )
                yb = ybs[tstart // 8]
                o8 = (tstart % 8) * 64
                if copy_eng == 0:
                    nc.vector.tensor_copy(yb[:, o8 : o8 + 64 * nch], py[:108, : 64 * nch])
                else:
                    nc.scalar.copy(yb[:, o8 : o8 + 64 * nch], py[:108, : 64 * nch])

            def emit_proj_out(half):
                yb = ybs[half]
                yv = y[864 * half : 864 * (half + 1), :].rearrange(
                    "(t i) c -> i t c", i=108
                )
                nc.sync.dma_start(out=yv, in_=yb[:].rearrange("p (t c) -> p t c", c=64))

            def emit_y_part(t0, t1):
                yv = y[t0:t1, :].rearrange("(t i) c -> i t c", i=108)
                c0 = (t0 // 108 - 8) * 64
                c1 = (t1 // 108 - 8) * 64
                nc.sync.dma_start(
                    out=yv,
                    in_=ybs[1][:, c0:c1].rearrange("p (t c) -> p t c", c=64))

            def qt_loop(qo, qn, hooks):
                pu = psu.tile([128, 512], F32, tag="u", name="pu")
                pend = []
                for pr in range(NPR):
                    sc = emit_S(qo, qn, pr)
                    # lag U by TWO iters so its es inputs are never fresh
                    # (U-group leader otherwise stalls on the exp engines)
                    if len(pend) == 2:
                        p0 = pend.pop(0)
                        emit_U(qn, p0[0], p0[1], pu)
                    hk = hooks.get(pr)
                    if hk:
                        hk()
                    es = emit_exp(qn, sc, 2)
                    pend.append((pr, es))
                for p0 in pend:
                    emit_U(qn, p0[0], p0[1], pu)
                return pu

            for qi, (qo, qn) in enumerate(QTS):
                hooks = {}
                if qi == 0:
                    hooks = {0: emit_vprime}
                elif qi == 1:
                    hooks = {4: lambda: emit_proj(0, 2, 0),
                             7: lambda: emit_proj(2, 2, 1)}
                elif qi == 2:
                    hooks = {4: lambda: emit_proj(4, 2, 0),
                             7: lambda: emit_proj(6, 2, 1),
                             10: lambda: emit_proj_out(0)}
                elif qi == 3:
                    hooks = {4: lambda: emit_proj(8, 2, 0),
                             7: lambda: emit_proj(10, 2, 1),
                             10: lambda: emit_proj(12, 2, 0),
                             12: lambda: emit_y_part(864, 1512)}
                pu = qt_loop(qo, qn, hooks)
                z = emit_divide(qo, qn, pu)
                if qi == 0 and debug:
                    # dump immediately: the dpool buffer is recycled by later qts
                    nc.sync.dma_start(out=d_u[:], in_=z[:])
            emit_proj(14, 2, 1)
            emit_y_part(1512, 1728)

            if debug:
                nc.sync.dma_start(out=d_qT[:], in_=qT[:])
                nc.sync.dma_start(out=d_kT[:], in_=kT[:])
                nc.sync.dma_start(out=d_vp[:], in_=vp[:])
                nc.sync.dma_start(out=d_ot[:], in_=ot[:])

    return nc


_NC = None


def _get_nc():
    global _NC
    if _NC is None:
        _NC = build_nc()
        _NC.finalize()
    return _NC


def make_in_maps(x, w_qkv, w_proj, b_proj):
    x2 = np.ascontiguousarray(x.reshape(C, N)).astype(ml_dtypes.bfloat16)
    in_maps = []
    for c in range(NCORES):
        sl = slice(8 * c, 8 * c + 8)
        w24 = np.concatenate(
            [
                w_qkv[sl, :].T,
                w_qkv[64 + 8 * c : 64 + 8 * c + 8, :].T,
                w_qkv[128 + 8 * c : 128 + 8 * c + 8, :].T,
            ],
            axis=1,
        ).astype(ml_dtypes.bfloat16)
        wp = np.concatenate(
            [w_proj[:, sl].T, (b_proj / NCORES)[None, :]], axis=0
        ).astype(np.float32)
        in_maps.append(
            {
                "x2": x2,
                "w24": np.ascontiguousarray(w24),
                "wp": np.ascontiguousarray(wp),
            }
        )
    return in_maps


def run(x, w_qkv, w_proj, b_proj, trace=False, **kw):
    nc = _get_nc()
    in_maps = make_in_maps(x, w_qkv, w_proj, b_proj)
    res = run_bass_kernel_spmd(
        nc, in_maps, core_ids=list(range(NCORES)), trace=trace, **kw
    )
    y = np.zeros((N, C), np.float32)
    for r in res.results:
        y += r["y"]
    return y.reshape(1, 12, 12, 12, C), res


def kernel(x, w_qkv, w_proj, b_proj):
    out, _ = run(
        np.asarray(x), np.asarray(w_qkv), np.asarray(w_proj), np.asarray(b_proj)
    )
    return out
